# revision 1
# baseline (speedup 1.0000x reference)
"""Trainium2 Bass kernel for nn_GPT3_56934086476265.

96-block GPT-style transformer, B=1, N=1024, FEAT=768, ATTN=128, VOCAB=32000.

Sharding (8 cores, 1 chip):
  - Embedding (x @ W_emb): vocab-contraction sharded; each core takes a 4096-wide
    vocab slice of x (columns) and W_emb (rows), computes a partial [1024,768],
    and a ReduceScatter sums the partials handing each core its 128-row
    sequence shard.
  - 96 blocks: sequence-parallel (128 seq rows per core). Per block one
    AllGather exchanges K^T|V (128x256 per rank) so every core attends over the
    full 1024-length sequence.
  - Out-projection + top-k: hidden state AllGathered once; each core computes
    logits^T for its 4096 vocab columns ([128 vocab x 1024 seq] tiles) and takes
    top-k along the sequence axis with max8 + match_replace + max8.

All matmuls run as float32r (tf32) with fp32 PSUM accumulation; everything else
(softmax, l2norm, residuals, top-k) is fp32.
"""

import math

import numpy as np

import concourse.bass as bass
import concourse.mybir as mybir
import concourse.tile as tile
from concourse.bass_utils import run_bass_kernel_spmd

N_CORES = 8
SEQ = 1024
FEAT = 768
ATTN = 128
NBLOCKS = 96
VOCAB = 32000
VP = 4096          # padded vocab per core (8*4096 = 32768 >= 32000)
SSH = 128          # sequence rows per core
NF = FEAT // 128   # 6 feature tiles
NVT = VP // 128    # 32 vocab tiles per core

dt = mybir.dt
F32 = dt.float32
F32R = dt.float32r
BF16 = dt.bfloat16
FP16 = dt.float16
FP8 = dt.float8e4
U8 = dt.uint8
KSC = 256.0
ADD = mybir.AluOpType.add
MULT = mybir.AluOpType.mult
AF = mybir.ActivationFunctionType
AX = mybir.AxisListType

N_WARM = 0

_WAITFIX_UID = [0]


def _split_excess_waits(nc, max_keep=1):
    """walrus codegen on this toolchain only encodes one attached sync-wait on
    several instruction formats (fp32 Matmult lowers to LDWEIGHTS with a single
    wait slot; Drain/NoOp similar). Move excess waits onto standalone
    EventSemaphore instructions just before each over-budget instruction."""
    n = 0
    for f in nc.m.functions:
        for b in f.blocks:
            insts = list(b.instructions)
            out = []
            changed = False
            for ins in insts:
                si = ins.sync_info
                if si is not None and si.on_wait and len(si.on_wait) > max_keep:
                    waits = list(si.on_wait)
                    excess, keep = waits[:-max_keep], waits[-max_keep:]
                    for w in excess:
                        _WAITFIX_UID[0] += 1
                        es = mybir.InstEventSemaphore(
                            name=f"I-waitfix-{_WAITFIX_UID[0]}", ins=[], outs=[]
                        )
                        es.engine = ins.engine
                        es.sync_info = mybir.SyncInfo(on_wait=[w], on_update=[])
                        out.append(es)
                        n += 1
                    ins.sync_info = mybir.SyncInfo(
                        on_wait=keep, on_update=si.on_update
                    )
                    changed = True
                out.append(ins)
            if changed:
                b.instructions = out
    return n


def _build(nblocks, rounds, with_bqkv, with_bo, with_b1, with_bout):
    nc = bass.Bass(num_devices=N_CORES)

    # ---- DRAM parameters (per-core data supplied through in_maps) ----
    x_sh = nc.declare_dram_parameter("x_sh", [SEQ, VP], F32, isOutput=False)
    wemb = nc.declare_dram_parameter("wemb", [VP, FEAT], F32, isOutput=False)
    wqkv = nc.declare_dram_parameter("wqkv", [FEAT, 3 * ATTN], F32, isOutput=False)
    wo = nc.declare_dram_parameter("wo", [ATTN, FEAT], F32, isOutput=False)
    w1 = nc.declare_dram_parameter("w1", [FEAT, FEAT], F32, isOutput=False)
    wout = nc.declare_dram_parameter("wout", [FEAT, VP], F32, isOutput=False)
    pe_i = nc.declare_dram_parameter("pe_i", [SSH, FEAT], F32, isOutput=False)
    ident = nc.declare_dram_parameter("ident", [128, 128], F32, isOutput=False)
    if with_bqkv:
        bqkv = nc.declare_dram_parameter("bqkv", [1, 3 * ATTN], F32, isOutput=False)
        ones1 = nc.declare_dram_parameter("ones1", [1, 128], F32, isOutput=False)
    if with_bo:
        bo_rep = nc.declare_dram_parameter("bo_rep", [128, FEAT], F32, isOutput=False)
    if with_b1:
        b1_rep = nc.declare_dram_parameter("b1_rep", [128, FEAT], F32, isOutput=False)
    if with_bout:
        bout_sh = nc.declare_dram_parameter("bout_sh", [NVT, 128], F32, isOutput=False)

    RW = 8 * rounds
    topv = nc.declare_dram_parameter("topv", [VP, RW], F32, isOutput=True)

    rg = [list(range(N_CORES))]
    fr = lambda ap: ap.bitcast(F32R)

    with tile.TileContext(nc) as tc:
        with (
            tc.tile_pool(name="const", bufs=1) as cpool,
            tc.tile_pool(name="psA", bufs=2, space="PSUM") as psA,
            tc.tile_pool(name="psB", bufs=2, space="PSUM") as psB,
            tc.tile_pool(name="psW", bufs=1, space="PSUM") as psW,
            tc.tile_pool(name="dram", bufs=2, space="DRAM") as dram,
        ):
            # ---- resident constants ----
            ident_sb = cpool.tile([128, 128], F32)
            nc.sync.dma_start(ident_sb[:], ident[:])
            ident_rsb = cpool.tile([128, 128], F32R)
            nc.sync.dma_start(ident_rsb[:], fr(ident[:]))
            ident_r = ident_rsb[:]
            pe_sb = cpool.tile([128, FEAT], F32)
            nc.sync.dma_start(pe_sb[:], pe_i[:])
            wqkv_sb = cpool.tile([128, NF * 384], F32R)
            nc.sync.dma_start(
                wqkv_sb.rearrange("p (t d) -> p t d", t=NF),
                fr(wqkv.rearrange("(t p) d -> p t d", p=128)),
            )
            w1_sb = cpool.tile([128, NF * FEAT], F32R)
            nc.sync.dma_start(
                w1_sb.rearrange("p (t d) -> p t d", t=NF),
                fr(w1.rearrange("(t p) d -> p t d", p=128)),
            )
            wo_sb = cpool.tile([128, FEAT], F32R)
            nc.sync.dma_start(wo_sb[:], fr(wo[:]))
            topv_all = cpool.tile([128, NVT * RW], F32)
            if with_bqkv:
                bqkv_sb = cpool.tile([1, 3 * ATTN], F32R)
                nc.sync.dma_start(bqkv_sb[:], fr(bqkv[:]))
                ones_sb = cpool.tile([1, 128], F32R)
                nc.sync.dma_start(ones_sb[:], fr(ones1[:]))
            if with_bo:
                bo_sb = cpool.tile([128, FEAT], F32)
                nc.sync.dma_start(bo_sb[:], bo_rep[:])
            if with_b1:
                b1_sb = cpool.tile([128, FEAT], F32)
                nc.sync.dma_start(b1_sb[:], b1_rep[:])
            if with_bout:
                bout_sb = cpool.tile([128, NVT], F32)
                nc.sync.dma_start(bout_sb[:], bout_sh.rearrange("c p -> p c"))

            # alternate PSUM->SBUF copies between DVE and ACT
            cp_i = [0]

            def cp(out_ap, in_ap):
                if cp_i[0] % 2 == 0:
                    nc.vector.tensor_copy(out_ap, in_ap)
                else:
                    nc.scalar.copy(out_ap, in_ap)
                cp_i[0] += 1

            MM = nc.tensor.matmul

            # h state persists across phases
            h_sb = cpool.tile([128, FEAT], F32, name="h_sb", tag="h_sb", bufs=2)

            # =========================== embedding ===========================
            rs_in = dram.tile([SEQ, FEAT], F32, bufs=1)
            rs_out = dram.tile([SSH, FEAT], F32, bufs=1)

            with tc.tile_pool(name="embw", bufs=1) as embw, tc.tile_pool(
                name="embx", bufs=2
            ) as embx:
                wemb_sb = embw.tile([128, NVT * FEAT], F32R)
                wr = fr(wemb.rearrange("(c p) f -> p c f", p=128))
                wsb = wemb_sb.rearrange("p (c f) -> p c f", c=NVT)
                for q in range(4):
                    nc.sync.dma_start(
                        wsb[:, 8 * q : 8 * (q + 1), :], wr[:, 8 * q : 8 * (q + 1), :]
                    )
                for t in range(SEQ // 128):
                    x_sb = embx.tile([128, VP], F32, name="x_sb", tag="x_sb")
                    nc.sync.dma_start(x_sb[:], x_sh[128 * t : 128 * (t + 1), :])
                    hp = psA.tile([128, 1024], F32, name="hp", tag="big")
                    for g in range(NVT // 4):
                        tpg = psB.tile([128, 512], F32, name="tpg", tag="small")
                        for u in range(4):
                            c = 4 * g + u
                            nc.tensor.transpose(
                                tpg[:, 128 * u : 128 * (u + 1)],
                                x_sb[:, 128 * c : 128 * (c + 1)],
                                ident_sb[:],
                            )
                        xT = embx.tile([128, 512], F32R, name="xT", tag="xT", bufs=3)
                        cp(xT[:], tpg[:])
                        for u in range(4):
                            c = 4 * g + u
                            MM(
                                hp[:, 0:512],
                                xT[:, 128 * u : 128 * (u + 1)],
                                wemb_sb[:, FEAT * c : FEAT * c + 512],
                                start=(c == 0),
                                stop=(c == NVT - 1),
                            )
                            MM(
                                hp[:, 512:768],
                                xT[:, 128 * u : 128 * (u + 1)],
                                wemb_sb[:, FEAT * c + 512 : FEAT * (c + 1)],
                                start=(c == 0),
                                stop=(c == NVT - 1),
                            )
                    hp_sb = embx.tile([128, FEAT], F32, name="hp_sb", tag="hp_sb")
                    cp(hp_sb[:], hp[:, 0:FEAT])
                    nc.sync.dma_start(rs_in[128 * t : 128 * (t + 1), :], hp_sb[:])

                nc.gpsimd.collective_compute(
                    "ReduceScatter", ADD, replica_groups=rg,
                    ins=[rs_in.opt()], outs=[rs_out.opt()],
                )
                h0_tmp = embx.tile([128, FEAT], F32, name="h0_tmp", tag="hp_sb")
                nc.sync.dma_start(h0_tmp[:], rs_out[:])
                nc.vector.tensor_tensor(h_sb[:], h0_tmp[:], pe_sb[:], ADD)

            # =========================== blocks ==============================
            # Per-block state carried in "raw" (unnormalized) form: m2_sb holds
            # the unnormalized block output X (h = X * rin2 rowwise), hT_raw its
            # transpose. Q|K|V are computed from X and scaled once by rin2
            # (linear fold). The first l2norm of each block cancels entirely
            # when b1 == 0: l2norm((n1pre + n1pre@W1)@W1) == l2norm(r2@W1).
            with tc.tile_pool(name="blk", bufs=2) as wk:
                hT_raw = None
                rin2 = None
                for blk in range(nblocks):
                    if blk == 0:
                        # bootstrap: treat h0 as X with scale 1
                        tpb = psA.tile([128, 1024], F32, name="tpb", tag="big")
                        for ft in range(NF):
                            nc.tensor.transpose(
                                tpb[:, 128 * ft : 128 * (ft + 1)],
                                h_sb[:, 128 * ft : 128 * (ft + 1)],
                                ident_sb[:],
                            )
                        hT_raw = wk.tile([128, FEAT], F32R, name="hT", tag="hT")
                        nc.vector.tensor_copy(hT_raw[:, 0:384], tpb[:, 0:384])
                        nc.scalar.copy(hT_raw[:, 384:768], tpb[:, 384:768])

                    # QKV_raw = X @ [Wq|Wk|Wv]; scale rows by rin2 -> true QKV
                    qkv = psB.tile([128, 384], F32, name="qkv", tag="small")
                    for ft in range(NF):
                        MM(
                            qkv[:, 0:384],
                            hT_raw[:, 128 * ft : 128 * (ft + 1)],
                            wqkv_sb[:, 384 * ft : 384 * (ft + 1)],
                            start=(ft == 0),
                            stop=(ft == NF - 1 and not with_bqkv),
                        )
                    if with_bqkv:
                        # bias is not scale-folded; only valid with blk-0 scale=1
                        MM(qkv[:, 0:384], ones_sb[:], bqkv_sb[:], start=False,
                           stop=True)
                    qkv_sb = wk.tile([128, 384], F32, name="qkv_sb", tag="qkv_sb")
                    if blk == 0:
                        nc.vector.tensor_copy(qkv_sb[:], qkv[:, 0:384])
                    else:
                        nc.vector.tensor_scalar_mul(qkv_sb[:], qkv[:, 0:384],
                                                    rin2[:])

                    # K^T (and Q^T) via PE transpose; V already in SBUF
                    tpk = psB.tile([128, 512], F32, name="tpk", tag="small")
                    nc.tensor.transpose(tpk[:, 0:128], qkv_sb[:, 128:256],
                                        ident_sb[:])
                    nc.tensor.transpose(tpk[:, 128:256], qkv_sb[:, 0:128],
                                        ident_sb[:])
                    kt_sb = wk.tile([128, 128], F32, name="kt_sb", tag="kt_sb")
                    nc.scalar.copy(kt_sb[:], tpk[:, 0:128])

                    # AllGather K^T | V across the 8 cores (two queues)
                    ag_in = dram.tile([128, 256], F32, name="ag_in", tag="ag_in")
                    nc.sync.dma_start(ag_in[:, 0:128], kt_sb[:])
                    nc.scalar.dma_start(ag_in[:, 128:256], qkv_sb[:, 256:384])
                    ag_out = dram.tile(
                        [N_CORES * 128, 256], F32, name="ag_out", tag="ag_out",
                        addr_space="Shared",
                    )
                    nc.gpsimd.collective_compute(
                        "AllGather", mybir.AluOpType.bypass, replica_groups=rg,
                        ins=[ag_in.opt()], outs=[ag_out.opt()],
                    )

                    # Q^T for the scores lhsT (off critical path, during AG)
                    qt_sb = wk.tile([128, 128], F32R, name="qt_sb", tag="qt_sb")
                    nc.vector.tensor_copy(qt_sb[:], tpk[:, 128:256])

                    # keep the PE HAM-warm while the collective is in flight
                    warm = psW.tile([128, 512], F32, name="warm", tag="warm")
                    for wix in range(24):
                        MM(warm[:], hT_raw[:, 0:128], w1_sb[:, 0:512])

                    ago = ag_out.rearrange("(j r) c -> r j c", r=128)
                    ktf = wk.tile([128, SEQ], F32R, name="ktf", tag="ktf")
                    vf = wk.tile([128, SEQ], F32R, name="vf", tag="vf")
                    ktf_r = ktf.rearrange("r (j m) -> r j m", j=N_CORES)
                    vf_r = vf.rearrange("r (j m) -> r j m", j=N_CORES)
                    nc.sync.dma_start(ktf_r[:, 0:4, :], fr(ago[:, 0:4, 0:128]))
                    nc.scalar.dma_start(vf_r[:, 0:4, :], fr(ago[:, 0:4, 128:256]))
                    nc.sync.dma_start(ktf_r[:, 4:8, :], fr(ago[:, 4:8, 0:128]))
                    nc.scalar.dma_start(vf_r[:, 4:8, :], fr(ago[:, 4:8, 128:256]))

                    # scores / softmax / P^T / AV, pipelined in two m-halves.
                    # Only block 0 needs the max-subtraction (unit-norm h keeps
                    # |S| < 1 afterwards), and runs unpipelined.
                    s_ps = psA.tile([128, 1024], F32, name="s_ps", tag="big")
                    p_sb = wk.tile([128, SEQ], F32, name="p_sb", tag="p_sb")
                    tpg2 = psA.tile([128, 1024], F32, name="tpg2", tag="big")
                    pt = wk.tile([128, SEQ], F32R, name="pt", tag="pt")
                    at_ps = psB.tile([128, 512], F32, name="at_ps", tag="small")
                    if blk == 0:
                        MM(s_ps[:, 0:512], qt_sb[:], ktf[:, 0:512])
                        MM(s_ps[:, 512:1024], qt_sb[:], ktf[:, 512:1024])
                        rowsum = wk.tile([128, 1], F32, name="rowsum", tag="sc3")
                        rowmax = wk.tile([128, 1], F32, name="rowmax", tag="sc1")
                        nc.vector.reduce_max(rowmax[:], s_ps[:], axis=AX.X)
                        negmax = wk.tile([128, 1], F32, name="negmax", tag="sc2")
                        nc.vector.tensor_scalar_mul(negmax[:], rowmax[:], -1.0)
                        nc.scalar.activation(
                            p_sb[:], s_ps[:], AF.Exp, bias=negmax[:],
                            accum_out=rowsum[:],
                        )
                        for j in range(8):
                            nc.tensor.transpose(
                                tpg2[:, 128 * j : 128 * (j + 1)],
                                p_sb[:, 128 * j : 128 * (j + 1)],
                                ident_sb[:],
                            )
                        nc.vector.tensor_copy(pt[:, 0:512], tpg2[:, 0:512])
                        nc.scalar.copy(pt[:, 512:1024], tpg2[:, 512:1024])
                        for j in range(8):
                            MM(
                                at_ps[:, 0:128],
                                vf[:, 128 * j : 128 * (j + 1)],
                                pt[:, 128 * j : 128 * (j + 1)],
                                start=(j == 0),
                                stop=(j == 7),
                            )
                    else:
                        rs0 = wk.tile([128, 1], F32, name="rs0", tag="sc1")
                        rs1 = wk.tile([128, 1], F32, name="rs1", tag="sc2")
                        MM(s_ps[:, 0:512], qt_sb[:], ktf[:, 0:512])
                        nc.scalar.activation(
                            p_sb[:, 0:512], s_ps[:, 0:512], AF.Exp,
                            accum_out=rs0[:],
                        )
                        MM(s_ps[:, 512:1024], qt_sb[:], ktf[:, 512:1024])
                        for j in range(4):
                            nc.tensor.transpose(
                                tpg2[:, 128 * j : 128 * (j + 1)],
                                p_sb[:, 128 * j : 128 * (j + 1)],
                                ident_sb[:],
                            )
                        nc.vector.tensor_copy(pt[:, 0:512], tpg2[:, 0:512])
                        nc.scalar.activation(
                            p_sb[:, 512:1024], s_ps[:, 512:1024], AF.Exp,
                            accum_out=rs1[:],
                        )
                        for j in range(4):
                            MM(
                                at_ps[:, 0:128],
                                vf[:, 128 * j : 128 * (j + 1)],
                                pt[:, 128 * j : 128 * (j + 1)],
                                start=(j == 0),
                                stop=False,
                            )
                        for j in range(4, 8):
                            nc.tensor.transpose(
                                tpg2[:, 128 * j : 128 * (j + 1)],
                                p_sb[:, 128 * j : 128 * (j + 1)],
                                ident_sb[:],
                            )
                        nc.scalar.copy(pt[:, 512:1024], tpg2[:, 512:1024])
                        for j in range(4, 8):
                            MM(
                                at_ps[:, 0:128],
                                vf[:, 128 * j : 128 * (j + 1)],
                                pt[:, 128 * j : 128 * (j + 1)],
                                start=False,
                                stop=(j == 7),
                            )
                        rowsum = wk.tile([128, 1], F32, name="rowsum", tag="sc3")
                        nc.vector.tensor_tensor(rowsum[:], rs0[:], rs1[:], ADD)
                    recip = wk.tile([128, 1], F32, name="recip", tag="sc4")
                    nc.vector.reciprocal(recip[:], rowsum[:])
                    at_sb = wk.tile([128, 128], F32R, name="at_sb", tag="at_sb")
                    nc.vector.tensor_copy(at_sb[:], at_ps[:, 0:128])

                    # o = A @ Wo -> [128 s, 768]
                    o_ps = psA.tile([128, 1024], F32, name="o_ps", tag="big")
                    MM(o_ps[:, 0:512], at_sb[:], wo_sb[:, 0:512])
                    MM(o_ps[:, 512:768], at_sb[:], wo_sb[:, 512:768])

                    # n1pre = h + o/Z (+bo); the first l2norm cancels unless b1
                    if not with_b1:
                        # m2 = (n1pre + n1pre@W1) @ W1 = m1 + m1@W1 with
                        # m1 = n1pre@W1 -- fold the residual add into the m2
                        # accumulation as an identity matmul.
                        n1pre = wk.tile([128, FEAT], F32R, name="n1pre",
                                        tag="n1pre")
                        nc.vector.scalar_tensor_tensor(
                            n1pre[:], o_ps[:, 0:FEAT], recip[:], h_sb[:],
                            op0=MULT, op1=ADD,
                        )
                        if with_bo:
                            n1pre2 = wk.tile([128, FEAT], F32R, name="n1pre2",
                                             tag="n1pre2")
                            nc.vector.tensor_tensor(n1pre2[:], n1pre[:],
                                                    bo_sb[:], ADD)
                            n1pre = n1pre2
                        tpn = psA.tile([128, 1024], F32R, name="tpn", tag="big")
                        for ft in range(NF):
                            nc.tensor.transpose(
                                tpn[:, 128 * ft : 128 * (ft + 1)],
                                n1pre[:, 128 * ft : 128 * (ft + 1)],
                                ident_r,
                            )
                        n1T = wk.tile([128, FEAT], F32R, name="n1T", tag="n1T")
                        nc.vector.tensor_copy(n1T[:, 0:384], tpn[:, 0:384])
                        nc.scalar.copy(n1T[:, 384:768], tpn[:, 384:768])

                        m1_ps = psA.tile([128, 1024], F32, name="m1_ps",
                                         tag="big")
                        for ft in range(NF):
                            MM(
                                m1_ps[:, 0:512],
                                n1T[:, 128 * ft : 128 * (ft + 1)],
                                w1_sb[:, FEAT * ft : FEAT * ft + 512],
                                start=(ft == 0),
                                stop=(ft == NF - 1),
                            )
                            MM(
                                m1_ps[:, 512:768],
                                n1T[:, 128 * ft : 128 * (ft + 1)],
                                w1_sb[:, FEAT * ft + 512 : FEAT * (ft + 1)],
                                start=(ft == 0),
                                stop=(ft == NF - 1),
                            )
                        m1_sb = wk.tile([128, FEAT], F32R, name="m1_sb",
                                        tag="m1_sb")
                        nc.vector.tensor_copy(m1_sb[:, 0:384], m1_ps[:, 0:384])
                        nc.scalar.copy(m1_sb[:, 384:768], m1_ps[:, 384:768])
                        tpr = psA.tile([128, 1024], F32R, name="tpr", tag="big")
                        for ft in range(NF):
                            nc.tensor.transpose(
                                tpr[:, 128 * ft : 128 * (ft + 1)],
                                m1_sb[:, 128 * ft : 128 * (ft + 1)],
                                ident_r,
                            )
                        m1T = wk.tile([128, FEAT], F32R, name="m1T", tag="r2T")
                        nc.vector.tensor_copy(m1T[:, 0:384], tpr[:, 0:384])
                        nc.scalar.copy(m1T[:, 384:768], tpr[:, 384:768])

                        m2_ps = psA.tile([128, 1024], F32, name="m2_ps",
                                         tag="big")
                        for ft in range(NF):
                            MM(
                                m2_ps[:, 0:512],
                                m1T[:, 128 * ft : 128 * (ft + 1)],
                                w1_sb[:, FEAT * ft : FEAT * ft + 512],
                                start=(ft == 0),
                                stop=False,
                            )
                            MM(
                                m2_ps[:, 512:768],
                                m1T[:, 128 * ft : 128 * (ft + 1)],
                                w1_sb[:, FEAT * ft + 512 : FEAT * (ft + 1)],
                                start=(ft == 0),
                                stop=False,
                            )
                        MM(m2_ps[:, 0:512], ident_r, m1_sb[:, 0:512],
                           start=False, stop=True)
                        MM(m2_ps[:, 512:768], ident_r, m1_sb[:, 512:768],
                           start=False, stop=True)
                    else:
                        n1pre0 = wk.tile([128, FEAT], F32, name="n1pre0",
                                         tag="n1pre")
                        nc.vector.scalar_tensor_tensor(
                            n1pre0[:], o_ps[:, 0:FEAT], recip[:], h_sb[:],
                            op0=MULT, op1=ADD,
                        )
                        n1pre = n1pre0
                        if with_bo:
                            n1pre2 = wk.tile([128, FEAT], F32, name="n1pre2",
                                             tag="n1pre2")
                            nc.vector.tensor_tensor(n1pre2[:], n1pre[:],
                                                    bo_sb[:], ADD)
                            n1pre = n1pre2
                        sq = wk.tile([128, FEAT], F32, name="sq", tag="sq")
                        ss1 = wk.tile([128, 1], F32, name="ss1", tag="sc5")
                        nc.scalar.activation(sq[:], n1pre[:], AF.Square,
                                             accum_out=ss1[:])
                        nrm1 = wk.tile([128, 1], F32, name="nrm1", tag="sc6")
                        nc.scalar.activation(nrm1[:], ss1[:], AF.Sqrt)
                        nrm1c = wk.tile([128, 1], F32, name="nrm1c", tag="sc6b")
                        nc.vector.tensor_scalar_max(nrm1c[:], nrm1[:], 1e-12)
                        rin1 = wk.tile([128, 1], F32, name="rin1", tag="sc7")
                        nc.vector.reciprocal(rin1[:], nrm1c[:])
                        n1s = wk.tile([128, FEAT], F32, name="n1s", tag="n1s")
                        nc.vector.tensor_scalar_mul(n1s[:], n1pre[:], rin1[:])

                        tpn = psA.tile([128, 1024], F32, name="tpn", tag="big")
                        for ft in range(NF):
                            nc.tensor.transpose(
                                tpn[:, 128 * ft : 128 * (ft + 1)],
                                n1s[:, 128 * ft : 128 * (ft + 1)],
                                ident_sb[:],
                            )
                        n1T = wk.tile([128, FEAT], F32R, name="n1T", tag="n1T")
                        nc.vector.tensor_copy(n1T[:, 0:384], tpn[:, 0:384])
                        nc.scalar.copy(n1T[:, 384:768], tpn[:, 384:768])
                        m1_ps = psA.tile([128, 1024], F32, name="m1_ps",
                                         tag="big")
                        for ft in range(NF):
                            MM(
                                m1_ps[:, 0:512],
                                n1T[:, 128 * ft : 128 * (ft + 1)],
                                w1_sb[:, FEAT * ft : FEAT * ft + 512],
                                start=(ft == 0),
                                stop=(ft == NF - 1),
                            )
                            MM(
                                m1_ps[:, 512:768],
                                n1T[:, 128 * ft : 128 * (ft + 1)],
                                w1_sb[:, FEAT * ft + 512 : FEAT * (ft + 1)],
                                start=(ft == 0),
                                stop=(ft == NF - 1),
                            )
                        r2 = wk.tile([128, FEAT], F32, name="r2", tag="r2")
                        nc.vector.tensor_tensor(r2[:], m1_ps[:, 0:FEAT], n1s[:],
                                                ADD)
                        r2b = wk.tile([128, FEAT], F32, name="r2b", tag="r2b")
                        nc.vector.tensor_tensor(r2b[:], r2[:], b1_sb[:], ADD)
                        tpr = psA.tile([128, 1024], F32, name="tpr", tag="big")
                        for ft in range(NF):
                            nc.tensor.transpose(
                                tpr[:, 128 * ft : 128 * (ft + 1)],
                                r2b[:, 128 * ft : 128 * (ft + 1)],
                                ident_sb[:],
                            )
                        r2T = wk.tile([128, FEAT], F32R, name="r2T", tag="r2T")
                        nc.vector.tensor_copy(r2T[:, 0:384], tpr[:, 0:384])
                        nc.scalar.copy(r2T[:, 384:768], tpr[:, 384:768])
                        m2_ps = psA.tile([128, 1024], F32, name="m2_ps",
                                         tag="big")
                        for ft in range(NF):
                            MM(
                                m2_ps[:, 0:512],
                                r2T[:, 128 * ft : 128 * (ft + 1)],
                                w1_sb[:, FEAT * ft : FEAT * ft + 512],
                                start=(ft == 0),
                                stop=(ft == NF - 1),
                            )
                            MM(
                                m2_ps[:, 512:768],
                                r2T[:, 128 * ft : 128 * (ft + 1)],
                                w1_sb[:, FEAT * ft + 512 : FEAT * (ft + 1)],
                                start=(ft == 0),
                                stop=(ft == NF - 1),
                            )

                    # h_new = l2norm(m2_raw (+ b1)): compute rin2 on the critical
                    # path; X copy + transpose + the h scale run alongside.
                    if with_b1:
                        hpre = wk.tile([128, FEAT], F32, name="hpre", tag="hpre")
                        nc.vector.tensor_tensor(hpre[:], m2_ps[:, 0:FEAT],
                                                b1_sb[:], ADD)
                        src = hpre[:]
                    else:
                        src = m2_ps[:, 0:FEAT]
                    ss2 = wk.tile([128, 1], F32, name="ss2", tag="sc5")
                    sq2 = wk.tile([128, FEAT], F32, name="sq2", tag="sq")
                    nc.scalar.activation(sq2[:], src, AF.Square, accum_out=ss2[:])
                    nrm2 = wk.tile([128, 1], F32, name="nrm2", tag="sc6")
                    nc.scalar.activation(nrm2[:], ss2[:], AF.Sqrt)
                    nrm2c = wk.tile([128, 1], F32, name="nrm2c", tag="sc6b")
                    nc.vector.tensor_scalar_max(nrm2c[:], nrm2[:], 1e-12)
                    rin2 = wk.tile([128, 1], F32, name="rin2", tag="sc7")
                    nc.vector.reciprocal(rin2[:], nrm2c[:])

                    # X (m2_sb), X^T, and h = X*rin2 for the next block
                    m2_sb = wk.tile([128, FEAT], F32, name="m2_sb", tag="m2_sb")
                    nc.vector.tensor_copy(m2_sb[:, 0:384], src[:, 0:384])
                    nc.scalar.copy(m2_sb[:, 384:768], src[:, 384:768])
                    tpb = psA.tile([128, 1024], F32, name="tpb", tag="big")
                    for ft in range(NF):
                        nc.tensor.transpose(
                            tpb[:, 128 * ft : 128 * (ft + 1)],
                            m2_sb[:, 128 * ft : 128 * (ft + 1)],
                            ident_sb[:],
                        )
                    hT_raw = wk.tile([128, FEAT], F32R, name="hT", tag="hT")
                    nc.vector.tensor_copy(hT_raw[:, 0:384], tpb[:, 0:384])
                    nc.scalar.copy(hT_raw[:, 384:768], tpb[:, 384:768])
                    h_sb = cpool.tile([128, FEAT], F32, name="h_sb", tag="h_sb",
                                      bufs=2)
                    nc.scalar.activation(h_sb[:], m2_sb[:], AF.Copy,
                                         scale=rin2[:])

                # final h^T for the out-projection, AllGathered to all cores
                tpf = psA.tile([128, 1024], F32, name="tpf", tag="big")
                for ft in range(NF):
                    nc.tensor.transpose(
                        tpf[:, 128 * ft : 128 * (ft + 1)],
                        h_sb[:, 128 * ft : 128 * (ft + 1)],
                        ident_sb[:],
                    )
                hTf = wk.tile([128, FEAT], F32, name="hTf", tag="hTf")
                nc.vector.tensor_copy(hTf[:, 0:384], tpf[:, 0:384])
                nc.scalar.copy(hTf[:, 384:768], tpf[:, 384:768])
                agh_in = dram.tile([FEAT, 128], F32, bufs=1)
                nc.sync.dma_start(
                    agh_in.rearrange("(t p) m -> p t m", p=128),
                    hTf.rearrange("p (t m) -> p t m", t=NF),
                )
                agh_out = dram.tile(
                    [N_CORES * FEAT, 128], F32, addr_space="Shared", bufs=1
                )
                nc.gpsimd.collective_compute(
                    "AllGather", mybir.AluOpType.bypass, replica_groups=rg,
                    ins=[agh_in.opt()], outs=[agh_out.opt()],
                )


            with tc.tile_pool(name="oph", bufs=2) as op:
                htf_sb = op.tile([128, NF * SEQ], F32R, name="htf_sb", tag="htf",
                                 bufs=1)
                agh_r = agh_out.rearrange("(j t p) m -> p t j m", t=NF, p=128)
                for ft in range(NF):
                    nc.sync.dma_start(
                        htf_sb[:, SEQ * ft : SEQ * (ft + 1)].rearrange(
                            "p (j m) -> p j m", j=N_CORES
                        ),
                        fr(agh_r[:, ft, :, :]),
                    )

                wout_r = wout.rearrange("(t p) v -> p t v", p=128)
                for c in range(NVT):
                    woc = op.tile([128, NF * 128], F32R, name="woc", tag="woc",
                                  bufs=3)
                    nc.sync.dma_start(
                        woc.rearrange("p (t v) -> p t v", t=NF),
                        fr(wout_r[:, :, 128 * c : 128 * (c + 1)]),
                    )
                    L_ps = psA.tile([128, 1024], F32, name="L_ps", tag="big")
                    for ft in range(NF):
                        MM(
                            L_ps[:, 0:512],
                            woc[:, 128 * ft : 128 * (ft + 1)],
                            htf_sb[:, SEQ * ft : SEQ * ft + 512],
                            start=(ft == 0),
                            stop=(ft == NF - 1),
                        )
                        MM(
                            L_ps[:, 512:1024],
                            woc[:, 128 * ft : 128 * (ft + 1)],
                            htf_sb[:, SEQ * ft + 512 : SEQ * (ft + 1)],
                            start=(ft == 0),
                            stop=(ft == NF - 1),
                        )
                    l_sb = op.tile([128, SEQ], F32, name="l_sb", tag="l_sb")
                    if with_bout:
                        nc.vector.tensor_scalar_add(
                            l_sb[:, 0:512], L_ps[:, 0:512], bout_sb[:, c : c + 1]
                        )
                        nc.vector.tensor_scalar_add(
                            l_sb[:, 512:1024], L_ps[:, 512:1024],
                            bout_sb[:, c : c + 1],
                        )
                    else:
                        nc.scalar.copy(l_sb[:, 0:512], L_ps[:, 0:512])
                        nc.scalar.copy(l_sb[:, 512:1024], L_ps[:, 512:1024])

                    nc.vector.max(topv_all[:, RW * c : RW * c + 8], l_sb[:])
                    prev = l_sb
                    for r in range(1, rounds):
                        mrb = op.tile(
                            [128, SEQ], F32, name="mrb", tag=f"mrb{r % 2}"
                        )
                        nc.vector.match_replace(
                            mrb[:],
                            topv_all[:, RW * c + 8 * (r - 1) : RW * c + 8 * r],
                            prev[:],
                            -1e30,
                        )
                        nc.vector.max(
                            topv_all[:, RW * c + 8 * r : RW * c + 8 * (r + 1)],
                            mrb[:],
                        )
                        prev = mrb

                nc.sync.dma_start(
                    topv.rearrange("(c p) w -> p c w", p=128),
                    topv_all.rearrange("p (c w) -> p c w", c=NVT),
                )

    _split_excess_waits(nc)
    return nc


def _build_fast(nblocks, rounds):
    """Fast path for the all-zero-bias case.

    Structural changes vs _build:
      - MLP collapse: with b1 == 0,
          h_new = l2norm((n1 + n1@W1) @ W1) = l2norm(n1pre @ (W1 + W1@W1))
        so one host-precomputed Wm replaces the m1/m2 two-matmul chain, and
          qkv_next_raw = n1pre @ (Wm @ Wqkv)   (host-precomputed Wmqkv)
        comes straight off n1pre^T. The l2norm scale rin2 is folded into K/V
        before the AllGather and into the exp() scale on the Q side.
      - The per-block K^T|V AllGather moves fp16 (512KB out vs 1MB); the
        attention inner ops (scores, P, AV, Wo) run on fp16 operands with
        fp32 PSUM accumulation.
      - x and W_emb arrive host-transposed/fp16, removing the embedding
        transposes and halving its DMA traffic.
      - The final h AllGather and the out-projection matmuls run in fp16.
    """
    nc = bass.Bass(num_devices=N_CORES)

    xT_h = nc.declare_dram_parameter("xT_h", [VP, SEQ], FP16, isOutput=False)
    wemb_h = nc.declare_dram_parameter("wemb_h", [VP, FEAT], FP16, isOutput=False)
    wqkv = nc.declare_dram_parameter("wqkv", [FEAT, 3 * ATTN], F32, isOutput=False)
    wm = nc.declare_dram_parameter("wm", [FEAT, FEAT], F32, isOutput=False)
    wmqkv = nc.declare_dram_parameter("wmqkv", [FEAT, 3 * ATTN], F32,
                                      isOutput=False)
    wo_bf = nc.declare_dram_parameter("wo_bf", [ATTN, FEAT], FP16, isOutput=False)
    wout_h = nc.declare_dram_parameter("wout_h", [FEAT, VP], FP16,
                                       isOutput=False)
    pe_i = nc.declare_dram_parameter("pe_i", [SSH, FEAT], F32, isOutput=False)
    ident = nc.declare_dram_parameter("ident", [128, 128], F32, isOutput=False)
    ident_b = nc.declare_dram_parameter("ident_b", [128, 128], FP16,
                                        isOutput=False)
    ones_c = nc.declare_dram_parameter("ones_c", [128, 1], F32, isOutput=False)

    RW = 8 * rounds
    topv = nc.declare_dram_parameter("topv", [VP, RW], FP16, isOutput=True)

    rg = [list(range(N_CORES))]
    fr = lambda ap: ap.bitcast(F32R)

    with tile.TileContext(nc) as tc:
        with (
            tc.tile_pool(name="const", bufs=1) as cpool,
            tc.tile_pool(name="psA", bufs=2, space="PSUM") as psA,
            tc.tile_pool(name="psB", bufs=2, space="PSUM") as psB,
            tc.tile_pool(name="psP", bufs=1, space="PSUM") as psP,
            tc.tile_pool(name="dram", bufs=2, space="DRAM") as dram,
        ):
            # ---- resident constants ----
            ident_sb = cpool.tile([128, 128], F32)
            nc.sync.dma_start(ident_sb[:], ident[:])
            ident_rsb = cpool.tile([128, 128], F32R)
            nc.sync.dma_start(ident_rsb[:], fr(ident[:]))
            ident_r = ident_rsb[:]
            ident_hsb = cpool.tile([128, 128], FP16)
            nc.sync.dma_start(ident_hsb[:], ident_b[:])
            pe_sb = cpool.tile([128, FEAT], F32)
            nc.sync.dma_start(pe_sb[:], pe_i[:])
            wqkv_sb = cpool.tile([128, NF * 384], F32R)
            wmqkv_sb = cpool.tile([128, NF * 384], F32R)
            wm_sb = cpool.tile([128, NF * FEAT], F32R)
            wo_sb = cpool.tile([128, FEAT], FP16)
            ones_sb = cpool.tile([128, 1], F32)
            nc.sync.dma_start(ones_sb[:], ones_c[:])
            topv_all = cpool.tile([128, NVT * RW], FP16)

            cp_i = [0]

            def cp(out_ap, in_ap):
                if cp_i[0] % 2 == 0:
                    nc.vector.tensor_copy(out_ap, in_ap)
                else:
                    nc.scalar.copy(out_ap, in_ap)
                cp_i[0] += 1

            MM = nc.tensor.matmul

            h_sb = cpool.tile([128, FEAT], F32, name="h_sb", tag="h_sb", bufs=2)

            # =========================== embedding ===========================
            rs_in = dram.tile([SEQ, FEAT], F32, bufs=1)
            rs_out = dram.tile([SSH, FEAT], F32, bufs=1)

            with tc.tile_pool(name="embw", bufs=1) as embw:
                wemb_sb = embw.tile([128, NVT * FEAT], FP16)
                wr = wemb_h.rearrange("(c p) f -> p c f", p=128)
                wsb = wemb_sb.rearrange("p (c f) -> p c f", c=NVT)
                for q in range(4):
                    eng = nc.sync if q % 2 == 0 else nc.scalar
                    eng.dma_start(
                        wsb[:, 8 * q : 8 * (q + 1), :], wr[:, 8 * q : 8 * (q + 1), :]
                    )
                # whole x^T resident in two fp16 tiles (16 vocab chunks each)
                xr = xT_h.rearrange("(c p) s -> p c s", p=128)
                xa = embw.tile([128, 16 * SEQ], FP16)
                xb = embw.tile([128, 16 * SEQ], FP16)
                xav = xa.rearrange("p (c s) -> p c s", c=16)
                xbv = xb.rearrange("p (c s) -> p c s", c=16)
                for q in range(4):
                    nc.sync.dma_start(
                        xav[:, 4 * q : 4 * (q + 1), :],
                        xr[:, 4 * q : 4 * (q + 1), :],
                    )
                    nc.scalar.dma_start(
                        xbv[:, 4 * q : 4 * (q + 1), :],
                        xr[:, 16 + 4 * q : 16 + 4 * (q + 1), :],
                    )
                # block-phase constants arrive behind the embedding data
                nc.sync.dma_start(
                    wqkv_sb.rearrange("p (t d) -> p t d", t=NF),
                    fr(wqkv.rearrange("(t p) d -> p t d", p=128)),
                )
                nc.sync.dma_start(
                    wmqkv_sb.rearrange("p (t d) -> p t d", t=NF),
                    fr(wmqkv.rearrange("(t p) d -> p t d", p=128)),
                )
                nc.scalar.dma_start(
                    wm_sb.rearrange("p (t d) -> p t d", t=NF),
                    fr(wm.rearrange("(t p) d -> p t d", p=128)),
                )
                nc.scalar.dma_start(wo_sb[:], wo_bf[:])
                for t in range(SEQ // 128):
                    hp = psA.tile([128, 1024], F32, name="hp", tag="big")
                    for c in range(NVT):
                        src = xa if c < 16 else xb
                        lhs = src[:, SEQ * (c % 16) + 128 * t :
                                  SEQ * (c % 16) + 128 * (t + 1)]
                        MM(
                            hp[:, 0:512],
                            lhs,
                            wemb_sb[:, FEAT * c : FEAT * c + 512],
                            start=(c == 0),
                            stop=(c == NVT - 1),
                        )
                        MM(
                            hp[:, 512:768],
                            lhs,
                            wemb_sb[:, FEAT * c + 512 : FEAT * (c + 1)],
                            start=(c == 0),
                            stop=(c == NVT - 1),
                        )
                    hp_sb = cpool.tile([128, FEAT], F32, name="hp_sb",
                                       tag="hp_sb", bufs=2)
                    cp(hp_sb[:], hp[:, 0:FEAT])
                    nc.sync.dma_start(rs_in[128 * t : 128 * (t + 1), :], hp_sb[:])

                nc.gpsimd.collective_compute(
                    "ReduceScatter", ADD, replica_groups=rg,
                    ins=[rs_in.opt()], outs=[rs_out.opt()],
                )
                h0_tmp = cpool.tile([128, FEAT], F32, name="h0_tmp", tag="hp_sb",
                                    bufs=2)
                nc.sync.dma_start(h0_tmp[:], rs_out[:])
                nc.vector.tensor_tensor(h_sb[:], h0_tmp[:], pe_sb[:], ADD)

            # =========================== blocks ==============================
            with tc.tile_pool(name="blk", bufs=2) as wk:
                q_ps = None
                rin2 = None
                m2_ps = None
                for blk in range(nblocks):
                    if blk == 0:
                        # bootstrap: qkv_raw = h0 @ Wqkv (scale 1)
                        tpb = psA.tile([128, 1024], F32, name="tpb", tag="big")
                        for ft in range(NF):
                            nc.tensor.transpose(
                                tpb[:, 128 * ft : 128 * (ft + 1)],
                                h_sb[:, 128 * ft : 128 * (ft + 1)],
                                ident_sb[:],
                            )
                        hT0 = wk.tile([128, FEAT], F32R, name="hT0", tag="n1T")
                        nc.vector.tensor_copy(hT0[:, 0:384], tpb[:, 0:384])
                        nc.scalar.copy(hT0[:, 384:768], tpb[:, 384:768])
                        q_ps = psB.tile([128, 512], F32, name="q_ps", tag="small")
                        for ft in range(NF):
                            MM(
                                q_ps[:, 0:384],
                                hT0[:, 128 * ft : 128 * (ft + 1)],
                                wqkv_sb[:, 384 * ft : 384 * (ft + 1)],
                                start=(ft == 0),
                                stop=(ft == NF - 1),
                            )
                        rin2 = ones_sb

                    # ---- pre-AG: K,V straight from q_ps PSUM, scaled ----
                    # blk0 ships fp16 K (sharp softmax); later blocks ship
                    # fp8-e4m3 K scaled by KSC (scores are tiny, fp8 is ample)
                    CW = 512 if blk == 0 else 384
                    kv_out = wk.tile([128, CW], U8, name="kv_out",
                                     tag=f"kv_out{min(blk, 1)}")
                    k_sc = wk.tile([128, 128], F32R, name="k_sc", tag="k_sc")
                    nc.vector.tensor_scalar_mul(k_sc[:], q_ps[:, 128:256],
                                                rin2[:])
                    nc.scalar.mul(kv_out[:, CW - 256 : CW].bitcast(FP16),
                                  q_ps[:, 256:384], rin2[:])
                    tpk = psB.tile([128, 512], F32, name="tpk", tag="small")
                    nc.tensor.transpose(tpk[:, 0:128].bitcast(F32R), k_sc[:],
                                        ident_r)
                    if blk == 0:
                        nc.vector.tensor_copy(
                            kv_out[:, 0:256].bitcast(FP16), tpk[:, 0:128])
                    else:
                        nc.vector.tensor_scalar_mul(
                            kv_out[:, 0:128].bitcast(FP8), tpk[:, 0:128], KSC)

                    ag_in = dram.tile([128, CW], U8, name="ag_in",
                                      tag=f"ag_in{min(blk, 1)}")
                    nc.sync.dma_start(ag_in[:], kv_out[:])
                    ag_out = dram.tile(
                        [N_CORES * 128, CW], U8, name="ag_out",
                        tag=f"ag_out{min(blk, 1)}", addr_space="Shared",
                    )
                    nc.gpsimd.collective_compute(
                        "AllGather", mybir.AluOpType.bypass, replica_groups=rg,
                        ins=[ag_in.opt()], outs=[ag_out.opt()],
                    )

                    # ---- during AG: Q^T, h update ----
                    qkv_q = wk.tile([128, 128], F32R, name="qkv_q", tag="qkv_q")
                    nc.vector.tensor_copy(qkv_q[:], q_ps[:, 0:128])
                    nc.tensor.transpose(tpk[:, 128:256].bitcast(F32R),
                                        qkv_q[:], ident_r)
                    if blk == 0:
                        qt_sb = wk.tile([128, 128], FP16, name="qt_sb",
                                        tag="qt_sb")
                        nc.vector.tensor_copy(qt_sb[:], tpk[:, 128:256])
                    else:
                        qt_sb = wk.tile([128, 128], FP8, name="qt_sb",
                                        tag="qt_sb")
                        nc.vector.tensor_scalar(qt_sb[:], tpk[:, 128:256],
                                                rin2[:], KSC, op0=MULT,
                                                op1=MULT)

                    if blk > 0:
                        # h_{blk} = m2_raw * rin2, straight from PSUM
                        h_sb = cpool.tile([128, FEAT], F32, name="h_sb",
                                          tag="h_sb", bufs=2)
                        nc.scalar.activation(h_sb[:], m2_ps[:, 0:FEAT], AF.Copy,
                                             scale=rin2[:])

                    # ---- post-AG: one interleaved K^T|V tile, 4 DMAs ----
                    ago = ag_out.rearrange("(j r) c -> r j c", r=128)
                    kv_all = wk.tile([128, 8 * CW], U8, name="kv_all",
                                     tag=f"kv_all{min(blk, 1)}")
                    kva = kv_all.rearrange("r (j c) -> r j c", j=N_CORES)
                    nc.sync.dma_start(kva[:, 0:2, :], ago[:, 0:2, :])
                    nc.scalar.dma_start(kva[:, 4:6, :], ago[:, 4:6, :])
                    nc.sync.dma_start(kva[:, 2:4, :], ago[:, 2:4, :])
                    nc.scalar.dma_start(kva[:, 6:8, :], ago[:, 6:8, :])
                    if blk == 0:
                        ktj = lambda j: kv_all[:, 512 * j : 512 * j + 256
                                               ].bitcast(FP16)
                        vfj = lambda j: kv_all[:, 512 * j + 256 : 512 * j + 512
                                               ].bitcast(FP16)
                    else:
                        ktj = lambda j: kv_all[:, 384 * j : 384 * j + 128
                                               ].bitcast(FP8)
                        vfj = lambda j: kv_all[:, 384 * j + 128 : 384 * j + 384
                                               ].bitcast(FP16)

                    # ---- scores / softmax / A^T, two m-halves pipelined ----
                    s_psA = psB.tile([128, 512], F32, name="s_psA",
                                     tag="small")
                    s_psB = psB.tile([128, 512], F32, name="s_psB",
                                     tag="small")
                    p_sb = wk.tile([128, SEQ], FP16, name="p_sb", tag="p_sb")
                    tpp = psP.tile([128, SEQ], FP16, name="tpp", tag="tpp")
                    pt = wk.tile([128, SEQ], FP16, name="pt", tag="pt")
                    at_ps = psB.tile([128, 512], F32, name="at_ps", tag="small")
                    if blk == 0:
                        for j in range(4):
                            MM(s_psA[:, 128 * j : 128 * (j + 1)], qt_sb[:],
                               ktj(j))
                        for j in range(4, 8):
                            MM(s_psB[:, 128 * (j - 4) : 128 * (j - 3)],
                               qt_sb[:], ktj(j))
                        rowmaxA = wk.tile([128, 1], F32, name="rowmaxA",
                                          tag="sc1")
                        rowmaxB = wk.tile([128, 1], F32, name="rowmaxB",
                                          tag="sc2")
                        nc.vector.reduce_max(rowmaxA[:], s_psA[:], axis=AX.X)
                        nc.vector.reduce_max(rowmaxB[:], s_psB[:], axis=AX.X)
                        rowmax = wk.tile([128, 1], F32, name="rowmax", tag="sc8")
                        nc.vector.tensor_tensor(rowmax[:], rowmaxA[:],
                                                rowmaxB[:],
                                                mybir.AluOpType.max)
                        negmax = wk.tile([128, 1], F32, name="negmax", tag="sc9")
                        nc.vector.tensor_scalar_mul(negmax[:], rowmax[:], -1.0)
                        rsA = wk.tile([128, 1], F32, name="rsA", tag="sc1")
                        rsB = wk.tile([128, 1], F32, name="rsB", tag="sc2")
                        nc.scalar.activation(
                            p_sb[:, 0:512], s_psA[:], AF.Exp, bias=negmax[:],
                            accum_out=rsA[:],
                        )
                        nc.scalar.activation(
                            p_sb[:, 512:1024], s_psB[:], AF.Exp, bias=negmax[:],
                            accum_out=rsB[:],
                        )
                        rowsum = wk.tile([128, 1], F32, name="rowsum", tag="sc3")
                        nc.vector.tensor_tensor(rowsum[:], rsA[:], rsB[:], ADD)
                        for j in range(8):
                            nc.tensor.transpose(
                                tpp[:, 128 * j : 128 * (j + 1)],
                                p_sb[:, 128 * j : 128 * (j + 1)],
                                ident_hsb[:],
                            )
                        nc.vector.tensor_copy(pt[:, 0:512], tpp[:, 0:512])
                        nc.scalar.copy(pt[:, 512:1024], tpp[:, 512:1024])
                        for j in range(8):
                            MM(
                                at_ps[:, 0:128],
                                vfj(j),
                                pt[:, 128 * j : 128 * (j + 1)],
                                start=(j == 0),
                                stop=(j == 7),
                            )
                    else:
                        rs0 = wk.tile([128, 1], F32, name="rs0", tag="sc1")
                        rs1 = wk.tile([128, 1], F32, name="rs1", tag="sc2")
                        for j in range(4):
                            MM(s_psA[:, 128 * j : 128 * (j + 1)], qt_sb[:],


# revision 10
# speedup vs baseline: 1.1768x; 1.1768x over previous
"""Trainium2 Bass kernel for nn_GPT3_56934086476265.

96-block GPT-style transformer, B=1, N=1024, FEAT=768, ATTN=128, VOCAB=32000.

Sharding (8 cores, 1 chip):
  - Embedding (x @ W_emb): vocab-contraction sharded; ReduceScatter hands each
    core its 128-row sequence shard.
  - 96 blocks: sequence-parallel (128 seq rows per core). Per block one
    AllGather exchanges K^T|V (both fp8, 128x256B per rank).
  - Out-projection + top-k: hidden state AllGathered once; each core computes
    logits^T for its 4096 vocab columns and takes top-k along the sequence
    axis with max8 + match_replace + max8.

Critical-path restructure (vs the earlier version): with all biases zero the
per-block recurrence
    n1pre_t = h_t + A~_t @ Wo,  X_t = n1pre_t @ Wm,  h_{t+1} = X_t / |X_t|
is evaluated as
    X_t        = P1 + (A~_t @ WoWm) / Z
    qkv_raw    = P2 + (A~_t @ WoWmWqkv) / Z         (payload for block t+1)
where P1 = h_t @ Wm and P2 = h_t @ WmWqkv are computed DURING the AllGather
window (PE is otherwise idle there), WoWm / WoWmWqkv are host-precomputed
rank-128 factors, and 1/Z (softmax normalizer) folds into the PSUM->SBUF
scalar_tensor_tensor. Only the attention + rank-128 matmuls + row-norm +
payload pack remain on the exposed post-AllGather chain.
"""

import math

import numpy as np

import concourse.bass as bass
import concourse.mybir as mybir
import concourse.tile as tile
from concourse.bass_utils import run_bass_kernel_spmd

N_CORES = 8
SEQ = 1024
FEAT = 768
ATTN = 128
NBLOCKS = 96
VOCAB = 32000
VP = 4096          # padded vocab per core (8*4096 = 32768 >= 32000)
SSH = 128          # sequence rows per core
NF = FEAT // 128   # 6 feature tiles
NVT = VP // 128    # 32 vocab tiles per core

dt = mybir.dt
F32 = dt.float32
F32R = dt.float32r
BF16 = dt.bfloat16
FP16 = dt.float16
FP8 = dt.float8e4
U8 = dt.uint8
KSC = 256.0
ADD = mybir.AluOpType.add
MULT = mybir.AluOpType.mult
AF = mybir.ActivationFunctionType
AX = mybir.AxisListType

_WAITFIX_UID = [0]


def _split_excess_waits(nc, max_keep=1):
    """walrus codegen on this toolchain only encodes one attached sync-wait on
    several instruction formats (fp32 Matmult lowers to LDWEIGHTS with a single
    wait slot; Drain/NoOp similar). Move excess waits onto standalone
    EventSemaphore instructions just before each over-budget instruction."""
    n = 0
    for f in nc.m.functions:
        for b in f.blocks:
            insts = list(b.instructions)
            out = []
            changed = False
            for ins in insts:
                si = ins.sync_info
                if si is not None and si.on_wait and len(si.on_wait) > max_keep:
                    waits = list(si.on_wait)
                    excess, keep = waits[:-max_keep], waits[-max_keep:]
                    for w in excess:
                        _WAITFIX_UID[0] += 1
                        es = mybir.InstEventSemaphore(
                            name=f"I-waitfix-{_WAITFIX_UID[0]}", ins=[], outs=[]
                        )
                        es.engine = ins.engine
                        es.sync_info = mybir.SyncInfo(on_wait=[w], on_update=[])
                        out.append(es)
                        n += 1
                    ins.sync_info = mybir.SyncInfo(
                        on_wait=keep, on_update=si.on_update
                    )
                    changed = True
                out.append(ins)
            if changed:
                b.instructions = out
    return n


def _build_fast2(nblocks, rounds):
    nc = bass.Bass(num_devices=N_CORES)

    xT_h = nc.declare_dram_parameter("xT_h", [VP, SEQ], FP16, isOutput=False)
    wemb_h = nc.declare_dram_parameter("wemb_h", [VP, FEAT], FP16, isOutput=False)
    wqkv = nc.declare_dram_parameter("wqkv", [FEAT, 3 * ATTN], F32, isOutput=False)
    wm = nc.declare_dram_parameter("wm", [FEAT, FEAT], F32, isOutput=False)
    wmqkv = nc.declare_dram_parameter("wmqkv", [FEAT, 3 * ATTN], F32,
                                      isOutput=False)
    wowm_h = nc.declare_dram_parameter("wowm_h", [ATTN, FEAT], FP16,
                                       isOutput=False)
    wowmq_h = nc.declare_dram_parameter("wowmq_h", [ATTN, 3 * ATTN], FP16,
                                        isOutput=False)
    wout_h = nc.declare_dram_parameter("wout_h", [FEAT, VP], FP16,
                                       isOutput=False)
    pe_i = nc.declare_dram_parameter("pe_i", [SSH, FEAT], F32, isOutput=False)
    ident = nc.declare_dram_parameter("ident", [128, 128], F32, isOutput=False)
    ident_b = nc.declare_dram_parameter("ident_b", [128, 128], FP16,
                                        isOutput=False)

    RW = 8 * rounds
    topv = nc.declare_dram_parameter("topv", [VP, RW], FP16, isOutput=True)

    rg = [list(range(N_CORES))]
    fr = lambda ap: ap.bitcast(F32R)

    with tile.TileContext(nc) as tc:
        with (
            tc.tile_pool(name="const", bufs=1) as cpool,
            tc.tile_pool(name="psA", bufs=2, space="PSUM") as psA,
            tc.tile_pool(name="psB", bufs=2, space="PSUM") as psB,
            tc.tile_pool(name="psP", bufs=1, space="PSUM") as psP,
            tc.tile_pool(name="psQ", bufs=1, space="PSUM") as psQ,
            tc.tile_pool(name="dram", bufs=2, space="DRAM") as dram,
        ):
            # ---- resident constants ----
            ident_sb = cpool.tile([128, 128], F32)
            nc.sync.dma_start(ident_sb[:], ident[:])
            ident_rsb = cpool.tile([128, 128], F32R)
            nc.sync.dma_start(ident_rsb[:], fr(ident[:]))
            ident_r = ident_rsb[:]
            ident_hsb = cpool.tile([128, 128], FP16)
            nc.sync.dma_start(ident_hsb[:], ident_b[:])
            pe_sb = cpool.tile([128, FEAT], F32)
            nc.sync.dma_start(pe_sb[:], pe_i[:])
            wqkv_sb = cpool.tile([128, NF * 384], F32R)
            wm_sb = cpool.tile([128, NF * FEAT], F32R)
            wmqkv_sb = cpool.tile([128, NF * 384], F32R)
            wowm_sb = cpool.tile([128, FEAT], FP16)
            wowmq_sb = cpool.tile([128, 384], FP16)
            topv_all = cpool.tile([128, NVT * RW], FP16)

            MM = nc.tensor.matmul

            # =========================== embedding ===========================
            rs_in = dram.tile([SEQ, FEAT], F32, bufs=1)
            rs_out = dram.tile([SSH, FEAT], F32, bufs=1)
            h0_sb = cpool.tile([128, FEAT], F32, name="h0_sb")

            with tc.tile_pool(name="embw", bufs=1) as embw:
                wemb_sb = embw.tile([128, NVT * FEAT], FP16)
                wr = wemb_h.rearrange("(c p) f -> p c f", p=128)
                wsb = wemb_sb.rearrange("p (c f) -> p c f", c=NVT)
                for q in range(4):
                    eng = nc.sync if q % 2 == 0 else nc.scalar
                    eng.dma_start(
                        wsb[:, 8 * q : 8 * (q + 1), :], wr[:, 8 * q : 8 * (q + 1), :]
                    )
                # whole x^T resident in two fp16 tiles (16 vocab chunks each)
                xr = xT_h.rearrange("(c p) s -> p c s", p=128)
                xa = embw.tile([128, 16 * SEQ], FP16)
                xb = embw.tile([128, 16 * SEQ], FP16)
                xav = xa.rearrange("p (c s) -> p c s", c=16)
                xbv = xb.rearrange("p (c s) -> p c s", c=16)
                for q in range(4):
                    nc.sync.dma_start(
                        xav[:, 4 * q : 4 * (q + 1), :],
                        xr[:, 4 * q : 4 * (q + 1), :],
                    )
                    nc.scalar.dma_start(
                        xbv[:, 4 * q : 4 * (q + 1), :],
                        xr[:, 16 + 4 * q : 16 + 4 * (q + 1), :],
                    )
                # block-phase constants arrive behind the embedding data
                nc.sync.dma_start(
                    wqkv_sb.rearrange("p (t d) -> p t d", t=NF),
                    fr(wqkv.rearrange("(t p) d -> p t d", p=128)),
                )
                nc.sync.dma_start(
                    wmqkv_sb.rearrange("p (t d) -> p t d", t=NF),
                    fr(wmqkv.rearrange("(t p) d -> p t d", p=128)),
                )
                nc.scalar.dma_start(
                    wm_sb.rearrange("p (t d) -> p t d", t=NF),
                    fr(wm.rearrange("(t p) d -> p t d", p=128)),
                )
                nc.scalar.dma_start(wowm_sb[:], wowm_h[:])
                nc.scalar.dma_start(wowmq_sb[:], wowmq_h[:])
                for t in range(SEQ // 128):
                    hp = psA.tile([128, 1024], F32, name="hp", tag="big")
                    for c in range(NVT):
                        src = xa if c < 16 else xb
                        lhs = src[:, SEQ * (c % 16) + 128 * t :
                                  SEQ * (c % 16) + 128 * (t + 1)]
                        MM(
                            hp[:, 0:512],
                            lhs,
                            wemb_sb[:, FEAT * c : FEAT * c + 512],
                            start=(c == 0),
                            stop=(c == NVT - 1),
                        )
                        MM(
                            hp[:, 512:768],
                            lhs,
                            wemb_sb[:, FEAT * c + 512 : FEAT * (c + 1)],
                            start=(c == 0),
                            stop=(c == NVT - 1),
                        )
                    hp_sb = cpool.tile([128, FEAT], F32, name="hp_sb",
                                       tag="hp_sb", bufs=2)
                    if t % 2 == 0:
                        nc.vector.tensor_copy(hp_sb[:], hp[:, 0:FEAT])
                    else:
                        nc.scalar.copy(hp_sb[:], hp[:, 0:FEAT])
                    nc.sync.dma_start(rs_in[128 * t : 128 * (t + 1), :], hp_sb[:])

                nc.gpsimd.collective_compute(
                    "ReduceScatter", ADD, replica_groups=rg,
                    ins=[rs_in.opt()], outs=[rs_out.opt()],
                )
                h0_tmp = cpool.tile([128, FEAT], F32, name="h0_tmp", tag="hp_sb",
                                    bufs=2)
                nc.sync.dma_start(h0_tmp[:], rs_out[:])
                nc.vector.tensor_tensor(h0_sb[:], h0_tmp[:], pe_sb[:], ADD)

            # =========================== blocks ==============================
            with tc.tile_pool(name="blk", bufs=2) as wk:
                at_sb = None
                recip = None
                P1 = None
                P2s = None
                rin2 = None
                X_sb = None
                qkv_raw = None
                qt = None
                kscale = None

                for blk in range(nblocks):
                    last = blk == nblocks - 1
                    if blk == 0:
                        # ---- bootstrap: qkv0 = h0 @ Wqkv, fp16 payload ----
                        tpb = psA.tile([128, 1024], F32, name="tpb0", tag="big")
                        for ft in range(NF):
                            nc.tensor.transpose(
                                tpb[:, 128 * ft : 128 * (ft + 1)],
                                h0_sb[:, 128 * ft : 128 * (ft + 1)],
                                ident_sb[:],
                            )
                        hT = wk.tile([128, FEAT], F32R, name="hT", tag="hT")
                        nc.vector.tensor_copy(hT[:, 0:384], tpb[:, 0:384])
                        nc.scalar.copy(hT[:, 384:768], tpb[:, 384:768])
                        q_ps = psB.tile([128, 512], F32, name="q_ps", tag="small")
                        for ft in range(NF):
                            MM(
                                q_ps[:, 0:384],
                                hT[:, 128 * ft : 128 * (ft + 1)],
                                wqkv_sb[:, 384 * ft : 384 * (ft + 1)],
                                start=(ft == 0),
                                stop=(ft == NF - 1),
                            )
                        qkv_sb = wk.tile([128, 384], F32, name="qkv_sb",
                                         tag="qkv_sb")
                        nc.vector.tensor_copy(qkv_sb[:], q_ps[:, 0:384])
                        tpk0 = psB.tile([128, 512], F32, name="tpk0",
                                        tag="small")
                        nc.tensor.transpose(tpk0[:, 0:128], qkv_sb[:, 128:256],
                                            ident_sb[:])
                        kv_out0 = wk.tile([128, 512], U8, name="kv_out0",
                                          tag="kv0")
                        nc.vector.tensor_copy(
                            kv_out0[:, 0:256].bitcast(FP16), tpk0[:, 0:128])
                        nc.scalar.copy(
                            kv_out0[:, 256:512].bitcast(FP16),
                            qkv_sb[:, 256:384])
                        ag_in = dram.tile([128, 512], U8, name="ag_in0",
                                          tag="agi0")
                        nc.sync.dma_start(ag_in[:], kv_out0[:])
                        ag_out = dram.tile(
                            [N_CORES * 128, 512], U8, name="ag_out0",
                            tag="ago0", addr_space="Shared",
                        )
                        nc.gpsimd.collective_compute(
                            "AllGather", mybir.AluOpType.bypass,
                            replica_groups=rg,
                            ins=[ag_in.opt()], outs=[ag_out.opt()],
                        )
                        # ---- during AG0: Q^T, P1/P2 from h0 ----
                        nc.tensor.transpose(tpk0[:, 128:256], qkv_sb[:, 0:128],
                                            ident_sb[:])
                        qt0 = wk.tile([128, 128], FP16, name="qt0", tag="qt0")
                        nc.vector.tensor_copy(qt0[:], tpk0[:, 128:256])
                        g_ps = psA.tile([128, 1024], F32, name="g_ps", tag="big")
                        for ft in range(NF):
                            MM(
                                g_ps[:, 0:512],
                                hT[:, 128 * ft : 128 * (ft + 1)],
                                wm_sb[:, FEAT * ft : FEAT * ft + 512],
                                start=(ft == 0),
                                stop=(ft == NF - 1),
                            )
                            MM(
                                g_ps[:, 512:768],
                                hT[:, 128 * ft : 128 * (ft + 1)],
                                wm_sb[:, FEAT * ft + 512 : FEAT * (ft + 1)],
                                start=(ft == 0),
                                stop=(ft == NF - 1),
                            )
                        p2_ps = psB.tile([128, 512], F32, name="p2_ps",
                                         tag="small")
                        for ft in range(NF):
                            MM(
                                p2_ps[:, 0:384],
                                hT[:, 128 * ft : 128 * (ft + 1)],
                                wmqkv_sb[:, 384 * ft : 384 * (ft + 1)],
                                start=(ft == 0),
                                stop=(ft == NF - 1),
                            )
                        P1 = wk.tile([128, FEAT], F32, name="P1", tag="P1")
                        nc.vector.tensor_copy(P1[:, 0:384], g_ps[:, 0:384])
                        nc.scalar.copy(P1[:, 384:768], g_ps[:, 384:768])
                        P2s = wk.tile([128, 384], F32, name="P2s", tag="P2s")
                        nc.scalar.copy(P2s[:], p2_ps[:, 0:384])

                        # ---- post-AG0: fp16 attention with max-subtract ----
                        ago = ag_out.rearrange("(j r) c -> r j c", r=128)
                        ktf0 = wk.tile([128, SEQ], FP16, name="ktf0", tag="ktf0")
                        vf0 = wk.tile([128, SEQ], FP16, name="vf0", tag="vf0")
                        nc.sync.dma_start(
                            ktf0.rearrange("r (j m) -> r j m", j=N_CORES),
                            ago[:, :, 0:256].bitcast(FP16),
                        )
                        nc.scalar.dma_start(
                            vf0.rearrange("r (j d) -> r j d", j=N_CORES),
                            ago[:, :, 256:512].bitcast(FP16),
                        )
                        s_psA = psB.tile([128, 512], F32, name="s_psA",
                                         tag="small")
                        s_psB = psB.tile([128, 512], F32, name="s_psB",
                                         tag="small")
                        MM(s_psA[:], qt0[:], ktf0[:, 0:512])
                        MM(s_psB[:], qt0[:], ktf0[:, 512:1024])
                        rmA = wk.tile([128, 1], F32, name="rmA", tag="sc1")
                        rmB = wk.tile([128, 1], F32, name="rmB", tag="sc2")
                        nc.vector.reduce_max(rmA[:], s_psA[:], axis=AX.X)
                        nc.vector.reduce_max(rmB[:], s_psB[:], axis=AX.X)
                        rowmax = wk.tile([128, 1], F32, name="rowmax", tag="sc8")
                        nc.vector.tensor_tensor(rowmax[:], rmA[:], rmB[:],
                                                mybir.AluOpType.max)
                        negmax = wk.tile([128, 1], F32, name="negmax", tag="sc9")
                        nc.vector.tensor_scalar_mul(negmax[:], rowmax[:], -1.0)
                        rs0 = wk.tile([128, 1], F32, name="rs0", tag="sc1")
                        rs1 = wk.tile([128, 1], F32, name="rs1", tag="sc2")
                        p_sb = wk.tile([128, SEQ], FP16, name="p_sb0",
                                       tag="p_sb0")
                        tpp = psP.tile([128, SEQ], FP16, name="tpp", tag="tpp")
                        pt0 = wk.tile([128, SEQ], FP16, name="pt0", tag="pt0")
                        at_ps = psB.tile([128, 512], F32, name="at_ps",
                                         tag="small")
                        nc.scalar.activation(
                            p_sb[:, 0:512], s_psA[:], AF.Exp, bias=negmax[:],
                            accum_out=rs0[:],
                        )
                        nc.scalar.activation(
                            p_sb[:, 512:1024], s_psB[:], AF.Exp, bias=negmax[:],
                            accum_out=rs1[:],
                        )
                        for j in range(8):
                            nc.tensor.transpose(
                                tpp[:, 128 * j : 128 * (j + 1)],
                                p_sb[:, 128 * j : 128 * (j + 1)],
                                ident_hsb[:],
                            )
                        nc.vector.tensor_copy(pt0[:, 0:512], tpp[:, 0:512])
                        nc.scalar.copy(pt0[:, 512:1024], tpp[:, 512:1024])
                        for j in range(8):
                            MM(
                                at_ps[:, 0:128],
                                vf0[:, 128 * j : 128 * (j + 1)],
                                pt0[:, 128 * j : 128 * (j + 1)],
                                start=(j == 0),
                                stop=(j == 7),
                            )
                        rowsum = wk.tile([128, 1], F32, name="rowsum", tag="sc3")
                        nc.vector.tensor_tensor(rowsum[:], rs0[:], rs1[:], ADD)
                        recip = wk.tile([128, 1], F32, name="recip", tag="sc4")
                        nc.vector.reciprocal(recip[:], rowsum[:])
                        at_sb = wk.tile([128, 128], FP16, name="at_sb",
                                        tag="at_sb")
                        nc.vector.tensor_copy(at_sb[:], at_ps[:, 0:128])
                    else:
                        # =================== steady-state block ===============
                        # pre-AG payload pack (uses qkv_raw, rin2 from blk-1)
                        ksc16 = wk.tile([128, 128], FP16, name="ksc16",
                                        tag="k16")
                        nc.gpsimd.tensor_scalar_mul(ksc16[:],
                                                    qkv_raw[:, 128:256],
                                                    kscale[:])
                        tpk = psQ.tile([128, 512], FP16, name="tpk", tag="tpk")
                        nc.tensor.transpose(tpk[:, 0:128], ksc16[:],
                                            ident_hsb[:])
                        kv_out = wk.tile([128, 256], U8, name="kv_out",
                                         tag="kvout")
                        nc.gpsimd.tensor_scalar_mul(
                            kv_out[:, 128:256].bitcast(FP8),
                            qkv_raw[:, 256:384], kscale[:])
                        nc.vector.tensor_copy(kv_out[:, 0:128].bitcast(FP8),
                                              tpk[:, 0:128])
                        ag_in = dram.tile([128, 256], U8, name="ag_in",
                                          tag="agi")
                        nc.sync.dma_start(ag_in[:], kv_out[:])
                        ag_out = dram.tile(
                            [N_CORES * 128, 256], U8, name="ag_out",
                            tag="ago", addr_space="Shared",
                        )
                        nc.gpsimd.collective_compute(
                            "AllGather", mybir.AluOpType.bypass,
                            replica_groups=rg,
                            ins=[ag_in.opt()], outs=[ag_out.opt()],
                        )

                        # ---- during AG: h, hT, P1/P2 for this block; Q^T ----
                        qs16 = wk.tile([128, 128], FP16, name="qs16",
                                       tag="q16")
                        nc.gpsimd.tensor_scalar_mul(qs16[:], qkv_raw[:, 0:128],
                                                    kscale[:])
                        nc.tensor.transpose(tpk[:, 128:256], qs16[:],
                                            ident_hsb[:])
                        qt = wk.tile([128, 128], FP8, name="qt", tag="qt")
                        nc.vector.tensor_copy(qt[:], tpk[:, 128:256])

                        h_sb = wk.tile([128, FEAT], F32R, name="h_sb",
                                       tag="h")
                        nc.scalar.activation(h_sb[:], X_sb[:], AF.Copy,
                                             scale=rin2[:])
                        tpb = psA.tile([128, 1024], F32R, name="tpb", tag="big")
                        for ft in range(NF):
                            nc.tensor.transpose(
                                tpb[:, 128 * ft : 128 * (ft + 1)],
                                h_sb[:, 128 * ft : 128 * (ft + 1)],
                                ident_r,
                            )
                        hT = wk.tile([128, FEAT], F32R, name="hT", tag="hT")
                        nc.vector.tensor_copy(hT[:, 0:384], tpb[:, 0:384])
                        nc.scalar.copy(hT[:, 384:768], tpb[:, 384:768])
                        g_ps = psA.tile([128, 1024], F32, name="g_ps",
                                        tag="big")
                        for ft in range(NF):
                            MM(
                                g_ps[:, 0:512],
                                hT[:, 128 * ft : 128 * (ft + 1)],
                                wm_sb[:, FEAT * ft : FEAT * ft + 512],
                                start=(ft == 0),
                                stop=(ft == NF - 1),
                            )
                            MM(
                                g_ps[:, 512:768],
                                hT[:, 128 * ft : 128 * (ft + 1)],
                                wm_sb[:, FEAT * ft + 512 : FEAT * (ft + 1)],
                                start=(ft == 0),
                                stop=(ft == NF - 1),
                            )
                        P1 = wk.tile([128, FEAT], F32, name="P1", tag="P1")
                        nc.vector.tensor_copy(P1[:, 0:384], g_ps[:, 0:384])
                        nc.scalar.copy(P1[:, 384:768], g_ps[:, 384:768])
                        if not last:
                            p2_ps = psB.tile([128, 512], F32, name="p2_ps",
                                             tag="small")
                            for ft in range(NF):
                                MM(
                                    p2_ps[:, 0:384],
                                    hT[:, 128 * ft : 128 * (ft + 1)],
                                    wmqkv_sb[:, 384 * ft : 384 * (ft + 1)],
                                    start=(ft == 0),
                                    stop=(ft == NF - 1),
                                )
                            P2s = wk.tile([128, 384], F32, name="P2s",
                                          tag="P2s")
                            nc.scalar.copy(P2s[:], p2_ps[:, 0:384])

                        # ---- post-AG: fp8 attention, no max-subtract ----
                        ago = ag_out.rearrange("(j r) c -> r j c", r=128)
                        ktf = wk.tile([128, SEQ], FP8, name="ktf", tag="ktf")
                        vf = wk.tile([128, SEQ], FP8, name="vf", tag="vf")
                        nc.sync.dma_start(
                            ktf.rearrange("r (j m) -> r j m", j=N_CORES),
                            ago[:, :, 0:128].bitcast(FP8),
                        )
                        nc.scalar.dma_start(
                            vf.rearrange("r (j d) -> r j d", j=N_CORES),
                            ago[:, :, 128:256].bitcast(FP8),
                        )
                        s_psA = psB.tile([128, 512], F32, name="s_psA",
                                         tag="small")
                        s_psB = psB.tile([128, 512], F32, name="s_psB",
                                         tag="small")
                        MM(s_psA[:], qt[:], ktf[:, 0:512])
                        MM(s_psB[:], qt[:], ktf[:, 512:1024])
                        rs0 = wk.tile([128, 1], F32, name="rs0", tag="sc1")
                        rs1 = wk.tile([128, 1], F32, name="rs1", tag="sc2")
                        p_sb = wk.tile([128, SEQ], FP16, name="p_sb",
                                       tag="p_sb")
                        tpp = psP.tile([128, SEQ], FP16, name="tpp", tag="tpp")
                        pt = wk.tile([128, SEQ], FP8, name="pt", tag="pt")
                        at_ps = psB.tile([128, 512], F32, name="at_ps",
                                         tag="small")
                        nc.scalar.activation(
                            p_sb[:, 0:512], s_psA[:], AF.Exp,
                            scale=1.0 / (KSC * KSC), accum_out=rs0[:],
                        )
                        for j in range(4):
                            nc.tensor.transpose(
                                tpp[:, 128 * j : 128 * (j + 1)],
                                p_sb[:, 128 * j : 128 * (j + 1)],
                                ident_hsb[:],
                            )
                        nc.vector.tensor_copy(pt[:, 0:512], tpp[:, 0:512])
                        nc.scalar.activation(
                            p_sb[:, 512:1024], s_psB[:], AF.Exp,
                            scale=1.0 / (KSC * KSC), accum_out=rs1[:],
                        )
                        for j in range(4, 8):
                            nc.tensor.transpose(
                                tpp[:, 128 * j : 128 * (j + 1)],
                                p_sb[:, 128 * j : 128 * (j + 1)],
                                ident_hsb[:],
                            )
                        nc.scalar.copy(pt[:, 512:1024],
                                       tpp[:, 512:1024])
                        for j in range(4):
                            MM(
                                at_ps[:, 0:128],
                                vf[:, 128 * j : 128 * (j + 1)],
                                pt[:, 128 * j : 128 * (j + 1)],
                                start=(j == 0),
                                stop=False,
                            )
                        for j in range(4, 8):
                            MM(
                                at_ps[:, 0:128],
                                vf[:, 128 * j : 128 * (j + 1)],
                                pt[:, 128 * j : 128 * (j + 1)],
                                start=False,
                                stop=(j == 7),
                            )
                        rowsum = wk.tile([128, 1], F32, name="rowsum",
                                         tag="sc3")
                        nc.vector.tensor_tensor(rowsum[:], rs0[:], rs1[:], ADD)
                        rsK = wk.tile([128, 1], F32, name="rsK", tag="sc8")
                        nc.vector.tensor_scalar_mul(rsK[:], rowsum[:], KSC)
                        recip = wk.tile([128, 1], F32, name="recip", tag="sc4")
                        nc.vector.reciprocal(recip[:], rsK[:])
                        at_sb = wk.tile([128, 128], FP16, name="at_sb",
                                        tag="at_sb")
                        nc.vector.tensor_copy(at_sb[:], at_ps[:, 0:128])

                    # ============ shared X / qkv_raw / rin2 update ============
                    x_ps = psA.tile([128, 1024], F32, name="x_ps", tag="big")
                    MM(x_ps[:, 0:512], at_sb[:], wowm_sb[:, 0:512])
                    MM(x_ps[:, 512:768], at_sb[:], wowm_sb[:, 512:768])
                    X_sb = wk.tile([128, FEAT], F32, name="X_sb", tag="X")
                    nc.vector.scalar_tensor_tensor(
                        X_sb[:, 0:384], x_ps[:, 0:384], recip[:],
                        P1[:, 0:384], op0=MULT, op1=ADD,
                    )
                    nc.vector.scalar_tensor_tensor(
                        X_sb[:, 384:768], x_ps[:, 384:768], recip[:],
                        P1[:, 384:768], op0=MULT, op1=ADD,
                    )
                    if not last:
                        q2_ps = psB.tile([128, 512], F32, name="q2_ps",
                                         tag="small")
                        MM(q2_ps[:, 0:384], at_sb[:], wowmq_sb[:])
                        qkv_raw = wk.tile([128, 384], F32, name="qkv_raw",
                                          tag="qraw")
                        nc.vector.scalar_tensor_tensor(
                            qkv_raw[:], q2_ps[:, 0:384], recip[:], P2s[:],
                            op0=MULT, op1=ADD,
                        )
                    # row norms: two halves so sq_a overlaps the second X half
                    sq2 = wk.tile([128, FEAT], F32, name="sq2", tag="sq")
                    ssa = wk.tile([128, 1], F32, name="ssa", tag="sc5")
                    ssb = wk.tile([128, 1], F32, name="ssb", tag="sc5b")
                    nc.scalar.activation(sq2[:, 0:384], X_sb[:, 0:384],
                                         AF.Square, accum_out=ssa[:])
                    nc.scalar.activation(sq2[:, 384:768], X_sb[:, 384:768],
                                         AF.Square, accum_out=ssb[:])
                    ss2 = wk.tile([128, 1], F32, name="ss2", tag="sc5c")
                    nc.vector.tensor_tensor(ss2[:], ssa[:], ssb[:], ADD)
                    nrm2 = wk.tile([128, 1], F32, name="nrm2", tag="sc6")
                    nc.scalar.activation(nrm2[:], ss2[:], AF.Sqrt)
                    nrm2c = wk.tile([128, 1], F32, name="nrm2c", tag="sc6b")
                    nc.vector.tensor_scalar_max(nrm2c[:], nrm2[:], 1e-12)
                    rin2 = wk.tile([128, 1], F32, name="rin2", tag="sc7")
                    nc.vector.reciprocal(rin2[:], nrm2c[:])
                    if not last:
                        kscale = wk.tile([128, 1], F32, name="kscale",
                                         tag="sc9")
                        nc.vector.tensor_scalar_mul(kscale[:], rin2[:], KSC)

                # ---- final h^T (fp16), AllGathered to all cores ----
                h_sb = wk.tile([128, FEAT], F32, name="h_sbf", tag="h")
                nc.scalar.activation(h_sb[:], X_sb[:], AF.Copy, scale=rin2[:])
                tpf = psA.tile([128, 1024], F32, name="tpf", tag="big")
                for ft in range(NF):
                    nc.tensor.transpose(
                        tpf[:, 128 * ft : 128 * (ft + 1)],
                        h_sb[:, 128 * ft : 128 * (ft + 1)],
                        ident_sb[:],
                    )
                hTf = wk.tile([128, FEAT], FP16, name="hTf", tag="hTf")
                nc.vector.tensor_copy(hTf[:, 0:384], tpf[:, 0:384])
                nc.scalar.copy(hTf[:, 384:768], tpf[:, 384:768])
                agh_in = dram.tile([FEAT, 128], FP16, bufs=1)
                nc.sync.dma_start(
                    agh_in.rearrange("(t p) m -> p t m", p=128),
                    hTf.rearrange("p (t m) -> p t m", t=NF),
                )
                agh_out = dram.tile(
                    [N_CORES * FEAT, 128], FP16, addr_space="Shared", bufs=1
                )
                nc.gpsimd.collective_compute(
                    "AllGather", mybir.AluOpType.bypass, replica_groups=rg,
                    ins=[agh_in.opt()], outs=[agh_out.opt()],
                )

            # ======================= out-projection ==========================
            with tc.tile_pool(name="oph", bufs=2) as op:
                htf_sb = op.tile([128, NF * SEQ], FP16, name="htf_sb", tag="htf",
                                 bufs=1)
                agh_r = agh_out.rearrange("(j t p) m -> p t j m", t=NF, p=128)
                for ft in range(NF):
                    nc.sync.dma_start(
                        htf_sb[:, SEQ * ft : SEQ * (ft + 1)].rearrange(
                            "p (j m) -> p j m", j=N_CORES
                        ),
                        agh_r[:, ft, :, :],
                    )

                wout_r = wout_h.rearrange("(t p) v -> p t v", p=128)
                for c in range(NVT):
                    woc = op.tile([128, NF * 128], FP16, name="woc", tag="woc",
                                  bufs=3)
                    # gpsimd queue: prefetch runs during the h AllGather
                    nc.gpsimd.dma_start(
                        woc.rearrange("p (t v) -> p t v", t=NF),
                        wout_r[:, :, 128 * c : 128 * (c + 1)],
                    )
                    L_ps = psA.tile([128, 1024], F32, name="L_ps", tag="big")
                    for ft in range(NF):
                        MM(
                            L_ps[:, 0:512],
                            woc[:, 128 * ft : 128 * (ft + 1)],
                            htf_sb[:, SEQ * ft : SEQ * ft + 512],
                            start=(ft == 0),
                            stop=(ft == NF - 1),
                        )
                        MM(
                            L_ps[:, 512:1024],
                            woc[:, 128 * ft : 128 * (ft + 1)],
                            htf_sb[:, SEQ * ft + 512 : SEQ * (ft + 1)],
                            start=(ft == 0),
                            stop=(ft == NF - 1),
                        )
                    l_sb = op.tile([128, SEQ], FP16, name="l_sb", tag="l_sb")
                    nc.scalar.copy(l_sb[:, 0:512], L_ps[:, 0:512])
                    nc.scalar.copy(l_sb[:, 512:1024], L_ps[:, 512:1024])

                    nc.vector.max(topv_all[:, RW * c : RW * c + 8], l_sb[:])
                    prev = l_sb
                    for r in range(1, rounds):
                        mrb = op.tile(
                            [128, SEQ], FP16, name="mrb", tag=f"mrb{r % 2}"
                        )
                        nc.vector.match_replace(
                            mrb[:],
                            topv_all[:, RW * c + 8 * (r - 1) : RW * c + 8 * r],
                            prev[:],
                            -60000.0,
                        )
                        nc.vector.max(
                            topv_all[:, RW * c + 8 * r : RW * c + 8 * (r + 1)],
                            mrb[:],
                        )
                        prev = mrb

                nc.sync.dma_start(
                    topv.rearrange("(c p) w -> p c w", p=128),
                    topv_all.rearrange("p (c w) -> p c w", c=NVT),
                )

    _split_excess_waits(nc)
    return nc


_CACHE = {}


def _get_program(nblocks, rounds):
    key = ("fast2", nblocks, rounds)
    if key not in _CACHE:
        _CACHE[key] = _build_fast2(nblocks, rounds)
    return _CACHE[key]


def kernel(x, pe, W_emb, b_emb, Wq, bq, Wk, bk, Wv, bv, Wo, bo, W1, b1, Wout,
           bout, k, _profile=False, _nblocks=NBLOCKS):
    x = np.asarray(x, dtype=np.float32).reshape(SEQ, VOCAB)
    pe = np.asarray(pe, dtype=np.float32)
    W_emb = np.asarray(W_emb, dtype=np.float32)
    Wq = np.asarray(Wq, dtype=np.float32)
    Wk = np.asarray(Wk, dtype=np.float32)
    Wv = np.asarray(Wv, dtype=np.float32)
    Wo = np.asarray(Wo, dtype=np.float32)
    W1 = np.asarray(W1, dtype=np.float32)
    Wout = np.asarray(Wout, dtype=np.float32)
    b_emb = np.asarray(b_emb, dtype=np.float32)
    bq = np.asarray(bq, dtype=np.float32)
    bk = np.asarray(bk, dtype=np.float32)
    bv = np.asarray(bv, dtype=np.float32)
    bo = np.asarray(bo, dtype=np.float32)
    b1 = np.asarray(b1, dtype=np.float32)
    bout = np.asarray(bout, dtype=np.float32)
    k = int(np.asarray(k))
    rounds = max(1, math.ceil(k / 8))
    assert rounds * 8 <= 24, f"k={k} too large for this kernel"
    assert not (np.any(bq) or np.any(bk) or np.any(bv) or np.any(bo)
                or np.any(b1) or np.any(bout)), "bias path not supported"

    nc = _get_program(_nblocks, rounds)

    # host-side shard prep
    VTOT = N_CORES * VP
    wemb_pad = np.zeros((VTOT, FEAT), dtype=np.float32)
    wemb_pad[:VOCAB, :] = W_emb
    wout_pad = np.zeros((FEAT, VTOT), dtype=np.float32)
    wout_pad[:, :VOCAB] = Wout
    wqkv = np.ascontiguousarray(np.concatenate([Wq, Wk, Wv], axis=1))
    ident = np.eye(128, dtype=np.float32)

    xT_pad = np.zeros((VTOT, SEQ), dtype=np.float32)
    xT_pad[:VOCAB, :] = x.T
    W1_64 = W1.astype(np.float64)
    Wm64 = W1_64 + W1_64 @ W1_64
    Wm = Wm64.astype(np.float32)
    Wmqkv64 = Wm64 @ wqkv.astype(np.float64)
    Wmqkv = Wmqkv64.astype(np.float32)
    Wo64 = Wo.astype(np.float64)
    WoWm = (Wo64 @ Wm64).astype(np.float32)
    WoWmqkv = (Wo64 @ Wmqkv64).astype(np.float32)
    ident_b = ident.astype(np.float16)

    in_maps = []
    for i in range(N_CORES):
        m = {
            "xT_h": np.ascontiguousarray(
                xT_pad[VP * i : VP * (i + 1), :]
            ).astype(np.float16),
            "wemb_h": np.ascontiguousarray(
                wemb_pad[VP * i : VP * (i + 1), :]
            ).astype(np.float16),
            "wqkv": wqkv,
            "wm": Wm,
            "wmqkv": Wmqkv,
            "wowm_h": WoWm.astype(np.float16),
            "wowmq_h": WoWmqkv.astype(np.float16),
            "wout_h": np.ascontiguousarray(
                wout_pad[:, VP * i : VP * (i + 1)]
            ).astype(np.float16),
            "pe_i": np.ascontiguousarray(
                pe[SSH * i : SSH * (i + 1), :] + b_emb
            ),
            "ident": ident,
            "ident_b": ident_b,
        }
        in_maps.append(m)

    res = None
    for attempt in range(3):
        try:
            res = run_bass_kernel_spmd(
                nc, in_maps, core_ids=list(range(N_CORES)), trace=_profile
            )
            break
        except Exception:
            # transient NRT/axon failures (e.g. NRT_EXEC_UNIT_UNRECOVERABLE)
            # have been observed; retry with the cached executable
            if attempt == 2:
                raise
            import time as _time
            _time.sleep(5)

    RW = 8 * rounds
    full = np.concatenate(
        [np.asarray(res.results[i]["topv"], dtype=np.float32).reshape(VP, RW)
         for i in range(N_CORES)], axis=0
    )
    vals = full[:VOCAB, :k]  # [VOCAB, k]
    out = np.ascontiguousarray(vals.T)[None, :, :]  # [1, k, VOCAB]

    if _profile:
        return out.astype(np.float32), res
    return out.astype(np.float32)


# revision 19
# speedup vs baseline: 1.1943x; 1.0149x over previous
"""Trainium2 Bass kernel for nn_GPT3_56934086476265.

96-block GPT-style transformer, B=1, N=1024, FEAT=768, ATTN=128, VOCAB=32000.

Sharding (8 cores, 1 chip):
  - Embedding (x @ W_emb): vocab-contraction sharded; ReduceScatter hands each
    core its 128-row sequence shard.
  - 96 blocks: sequence-parallel (128 seq rows per core). Per block one
    AllGather exchanges K^T|V (both fp8, 128x256B per rank).
  - Out-projection + top-k: hidden state AllGathered once; each core computes
    logits^T for its 4096 vocab columns and takes top-k along the sequence
    axis with max8 + match_replace + max8.

Critical-path restructure (vs the earlier version): with all biases zero the
per-block recurrence
    n1pre_t = h_t + A~_t @ Wo,  X_t = n1pre_t @ Wm,  h_{t+1} = X_t / |X_t|
is evaluated as
    X_t        = P1 + (A~_t @ WoWm) / Z
    qkv_raw    = P2 + (A~_t @ WoWmWqkv) / Z         (payload for block t+1)
where P1 = h_t @ Wm and P2 = h_t @ WmWqkv are computed DURING the AllGather
window (PE is otherwise idle there), WoWm / WoWmWqkv are host-precomputed
rank-128 factors, and 1/Z (softmax normalizer) folds into the PSUM->SBUF
scalar_tensor_tensor. Only the attention + rank-128 matmuls + row-norm +
payload pack remain on the exposed post-AllGather chain.
"""

import math

import numpy as np

import concourse.bass as bass
import concourse.mybir as mybir
import concourse.tile as tile
from concourse.bass_utils import run_bass_kernel_spmd

N_CORES = 8
SEQ = 1024
FEAT = 768
ATTN = 128
NBLOCKS = 96
VOCAB = 32000
VP = 4096          # padded vocab per core (8*4096 = 32768 >= 32000)
SSH = 128          # sequence rows per core
NF = FEAT // 128   # 6 feature tiles
NVT = VP // 128    # 32 vocab tiles per core

dt = mybir.dt
F32 = dt.float32
F32R = dt.float32r
BF16 = dt.bfloat16
FP16 = dt.float16
FP8 = dt.float8e4
U8 = dt.uint8
KSC = 256.0
ADD = mybir.AluOpType.add
MULT = mybir.AluOpType.mult
AF = mybir.ActivationFunctionType
AX = mybir.AxisListType

_WAITFIX_UID = [0]


def _split_excess_waits(nc, max_keep=1):
    """walrus codegen on this toolchain only encodes one attached sync-wait on
    several instruction formats (fp32 Matmult lowers to LDWEIGHTS with a single
    wait slot; Drain/NoOp similar). Move excess waits onto standalone
    EventSemaphore instructions just before each over-budget instruction."""
    n = 0
    for f in nc.m.functions:
        for b in f.blocks:
            insts = list(b.instructions)
            out = []
            changed = False
            for ins in insts:
                si = ins.sync_info
                if si is not None and si.on_wait and len(si.on_wait) > max_keep:
                    waits = list(si.on_wait)
                    excess, keep = waits[:-max_keep], waits[-max_keep:]
                    for w in excess:
                        _WAITFIX_UID[0] += 1
                        es = mybir.InstEventSemaphore(
                            name=f"I-waitfix-{_WAITFIX_UID[0]}", ins=[], outs=[]
                        )
                        es.engine = ins.engine
                        es.sync_info = mybir.SyncInfo(on_wait=[w], on_update=[])
                        out.append(es)
                        n += 1
                    ins.sync_info = mybir.SyncInfo(
                        on_wait=keep, on_update=si.on_update
                    )
                    changed = True
                out.append(ins)
            if changed:
                b.instructions = out
    return n


def _build_fast2(nblocks, rounds):
    nc = bass.Bass(num_devices=N_CORES)

    xT_h = nc.declare_dram_parameter("xT_h", [VP, SEQ], FP16, isOutput=False)
    wemb_h = nc.declare_dram_parameter("wemb_h", [VP, FEAT], FP16, isOutput=False)
    wqkv = nc.declare_dram_parameter("wqkv", [FEAT, 3 * ATTN], F32, isOutput=False)
    wm = nc.declare_dram_parameter("wm", [FEAT, FEAT], F32, isOutput=False)
    wmqkv = nc.declare_dram_parameter("wmqkv", [FEAT, 3 * ATTN], F32,
                                      isOutput=False)
    wowm_h = nc.declare_dram_parameter("wowm_h", [ATTN, FEAT], FP16,
                                       isOutput=False)
    wowmq_h = nc.declare_dram_parameter("wowmq_h", [ATTN, 3 * ATTN], FP16,
                                        isOutput=False)
    wout_h = nc.declare_dram_parameter("wout_h", [FEAT, VP], FP16,
                                       isOutput=False)
    pe_i = nc.declare_dram_parameter("pe_i", [SSH, FEAT], F32, isOutput=False)
    ident = nc.declare_dram_parameter("ident", [128, 128], F32, isOutput=False)
    ident_b = nc.declare_dram_parameter("ident_b", [128, 128], FP16,
                                        isOutput=False)

    RW = 8 * rounds
    topv = nc.declare_dram_parameter("topv", [VP, RW], FP16, isOutput=True)

    rg = [list(range(N_CORES))]
    fr = lambda ap: ap.bitcast(F32R)

    with tile.TileContext(nc) as tc:
        with (
            tc.tile_pool(name="const", bufs=1) as cpool,
            tc.tile_pool(name="psA", bufs=2, space="PSUM") as psA,
            tc.tile_pool(name="psB", bufs=2, space="PSUM") as psB,
            tc.tile_pool(name="psQ", bufs=1, space="PSUM") as psQ,
            tc.tile_pool(name="dram", bufs=2, space="DRAM") as dram,
        ):
            # ---- resident constants ----
            ident_sb = cpool.tile([128, 128], F32)
            nc.sync.dma_start(ident_sb[:], ident[:])
            ident_rsb = cpool.tile([128, 128], F32R)
            nc.sync.dma_start(ident_rsb[:], fr(ident[:]))
            ident_r = ident_rsb[:]
            ident_hsb = cpool.tile([128, 128], FP16)
            nc.sync.dma_start(ident_hsb[:], ident_b[:])
            ones_f8 = cpool.tile([128, 1], FP16)
            nc.vector.memset(ones_f8[:], KSC)
            pe_sb = cpool.tile([128, FEAT], F32)
            nc.sync.dma_start(pe_sb[:], pe_i[:])
            wqkv_sb = cpool.tile([128, NF * 384], F32R)
            wm_sb = cpool.tile([128, NF * FEAT], F32R)
            wmqkv_sb = cpool.tile([128, NF * 384], F32R)
            wowm_sb = cpool.tile([128, FEAT], FP16)
            wowmq_sb = cpool.tile([128, 384], FP16)
            topv_all = cpool.tile([128, NVT * RW], FP16)

            MM = nc.tensor.matmul

            # =========================== embedding ===========================
            rs_in = dram.tile([SEQ, FEAT], F32, bufs=1)
            rs_out = dram.tile([SSH, FEAT], F32, bufs=1)
            h0_sb = cpool.tile([128, FEAT], F32, name="h0_sb")

            with tc.tile_pool(name="embw", bufs=1) as embw:
                wemb_sb = embw.tile([128, NVT * FEAT], FP16)
                wr = wemb_h.rearrange("(c p) f -> p c f", p=128)
                wsb = wemb_sb.rearrange("p (c f) -> p c f", c=NVT)
                for q in range(4):
                    eng = nc.sync if q % 2 == 0 else nc.scalar
                    eng.dma_start(
                        wsb[:, 8 * q : 8 * (q + 1), :], wr[:, 8 * q : 8 * (q + 1), :]
                    )
                # whole x^T resident in two fp16 tiles (16 vocab chunks each)
                xr = xT_h.rearrange("(c p) s -> p c s", p=128)
                xa = embw.tile([128, 16 * SEQ], FP16)
                xb = embw.tile([128, 16 * SEQ], FP16)
                xav = xa.rearrange("p (c s) -> p c s", c=16)
                xbv = xb.rearrange("p (c s) -> p c s", c=16)
                for q in range(4):
                    nc.sync.dma_start(
                        xav[:, 4 * q : 4 * (q + 1), :],
                        xr[:, 4 * q : 4 * (q + 1), :],
                    )
                    nc.scalar.dma_start(
                        xbv[:, 4 * q : 4 * (q + 1), :],
                        xr[:, 16 + 4 * q : 16 + 4 * (q + 1), :],
                    )
                # block-phase constants arrive behind the embedding data
                nc.sync.dma_start(
                    wqkv_sb.rearrange("p (t d) -> p t d", t=NF),
                    fr(wqkv.rearrange("(t p) d -> p t d", p=128)),
                )
                nc.sync.dma_start(
                    wmqkv_sb.rearrange("p (t d) -> p t d", t=NF),
                    fr(wmqkv.rearrange("(t p) d -> p t d", p=128)),
                )
                nc.scalar.dma_start(
                    wm_sb.rearrange("p (t d) -> p t d", t=NF),
                    fr(wm.rearrange("(t p) d -> p t d", p=128)),
                )
                nc.scalar.dma_start(wowm_sb[:], wowm_h[:])
                nc.scalar.dma_start(wowmq_sb[:], wowmq_h[:])
                for t in range(SEQ // 128):
                    hp = psA.tile([128, 1024], F32, name="hp", tag="big")
                    for c in range(NVT):
                        src = xa if c < 16 else xb
                        lhs = src[:, SEQ * (c % 16) + 128 * t :
                                  SEQ * (c % 16) + 128 * (t + 1)]
                        MM(
                            hp[:, 0:512],
                            lhs,
                            wemb_sb[:, FEAT * c : FEAT * c + 512],
                            start=(c == 0),
                            stop=(c == NVT - 1),
                        )
                        MM(
                            hp[:, 512:768],
                            lhs,
                            wemb_sb[:, FEAT * c + 512 : FEAT * (c + 1)],
                            start=(c == 0),
                            stop=(c == NVT - 1),
                        )
                    hp_sb = cpool.tile([128, FEAT], F32, name="hp_sb",
                                       tag="hp_sb", bufs=2)
                    if t % 2 == 0:
                        nc.vector.tensor_copy(hp_sb[:], hp[:, 0:FEAT])
                    else:
                        nc.scalar.copy(hp_sb[:], hp[:, 0:FEAT])
                    nc.sync.dma_start(rs_in[128 * t : 128 * (t + 1), :], hp_sb[:])

                nc.gpsimd.collective_compute(
                    "ReduceScatter", ADD, replica_groups=rg,
                    ins=[rs_in.opt()], outs=[rs_out.opt()],
                )
                h0_tmp = cpool.tile([128, FEAT], F32, name="h0_tmp", tag="hp_sb",
                                    bufs=2)
                nc.sync.dma_start(h0_tmp[:], rs_out[:])
                nc.vector.tensor_tensor(h0_sb[:], h0_tmp[:], pe_sb[:], ADD)

            # =========================== blocks ==============================
            with tc.tile_pool(name="blk", bufs=2) as wk:
                at_sb = None
                recip = None
                P1 = None
                P2s = None
                rin2 = None
                X_sb = None
                qkv_raw = None
                qt = None
                kscale = None

                for blk in range(nblocks):
                    last = blk == nblocks - 1
                    if blk == 0:
                        # ---- bootstrap: qkv0 = h0 @ Wqkv, fp16 payload ----
                        tpb = psA.tile([128, 1024], F32, name="tpb0", tag="big")
                        for ft in range(NF):
                            nc.tensor.transpose(
                                tpb[:, 128 * ft : 128 * (ft + 1)],
                                h0_sb[:, 128 * ft : 128 * (ft + 1)],
                                ident_sb[:],
                            )
                        hT = wk.tile([128, FEAT], F32R, name="hT", tag="hT")
                        nc.vector.tensor_copy(hT[:, 0:384], tpb[:, 0:384])
                        nc.scalar.copy(hT[:, 384:768], tpb[:, 384:768])
                        q_ps = psB.tile([128, 512], F32, name="q_ps", tag="small")
                        for ft in range(NF):
                            MM(
                                q_ps[:, 0:384],
                                hT[:, 128 * ft : 128 * (ft + 1)],
                                wqkv_sb[:, 384 * ft : 384 * (ft + 1)],
                                start=(ft == 0),
                                stop=(ft == NF - 1),
                            )
                        qkv_sb = wk.tile([128, 384], F32, name="qkv_sb",
                                         tag="qkv_sb")
                        nc.vector.tensor_copy(qkv_sb[:], q_ps[:, 0:384])
                        tpk0 = psB.tile([128, 512], F32, name="tpk0",
                                        tag="small")
                        nc.tensor.transpose(tpk0[:, 0:128], qkv_sb[:, 128:256],
                                            ident_sb[:])
                        kv_out0 = wk.tile([128, 512], U8, name="kv_out0",
                                          tag="kv0")
                        nc.vector.tensor_copy(
                            kv_out0[:, 0:256].bitcast(FP16), tpk0[:, 0:128])
                        nc.scalar.copy(
                            kv_out0[:, 256:512].bitcast(FP16),
                            qkv_sb[:, 256:384])
                        ag_in = dram.tile([128, 512], U8, name="ag_in0",
                                          tag="agi0")
                        nc.sync.dma_start(ag_in[:], kv_out0[:])
                        ag_out = dram.tile(
                            [N_CORES * 128, 512], U8, name="ag_out0",
                            tag="ago0", addr_space="Shared",
                        )
                        nc.gpsimd.collective_compute(
                            "AllGather", mybir.AluOpType.bypass,
                            replica_groups=rg,
                            ins=[ag_in.opt()], outs=[ag_out.opt()],
                        )
                        # ---- during AG0: Q^T, P1/P2 from h0 ----
                        nc.tensor.transpose(tpk0[:, 128:256], qkv_sb[:, 0:128],
                                            ident_sb[:])
                        qt0 = wk.tile([128, 128], FP16, name="qt0", tag="qt0")
                        nc.vector.tensor_copy(qt0[:], tpk0[:, 128:256])
                        g_ps = psA.tile([128, 1024], F32, name="g_ps", tag="big")
                        for ft in range(NF):
                            MM(
                                g_ps[:, 0:512],
                                hT[:, 128 * ft : 128 * (ft + 1)],
                                wm_sb[:, FEAT * ft : FEAT * ft + 512],
                                start=(ft == 0),
                                stop=(ft == NF - 1),
                            )
                            MM(
                                g_ps[:, 512:768],
                                hT[:, 128 * ft : 128 * (ft + 1)],
                                wm_sb[:, FEAT * ft + 512 : FEAT * (ft + 1)],
                                start=(ft == 0),
                                stop=(ft == NF - 1),
                            )
                        p2_ps = psB.tile([128, 512], F32, name="p2_ps",
                                         tag="small")
                        for ft in range(NF):
                            MM(
                                p2_ps[:, 0:384],
                                hT[:, 128 * ft : 128 * (ft + 1)],
                                wmqkv_sb[:, 384 * ft : 384 * (ft + 1)],
                                start=(ft == 0),
                                stop=(ft == NF - 1),
                            )
                        P1 = wk.tile([128, FEAT], F32, name="P1", tag="P1")
                        nc.vector.tensor_copy(P1[:, 0:384], g_ps[:, 0:384])
                        nc.scalar.copy(P1[:, 384:768], g_ps[:, 384:768])
                        P2s = wk.tile([128, 384], F32, name="P2s", tag="P2s")
                        nc.scalar.copy(P2s[:], p2_ps[:, 0:384])

                        # ---- post-AG0: fp16 attention with max-subtract ----
                        ago = ag_out.rearrange("(j r) c -> r j c", r=128)
                        ktf0 = wk.tile([128, SEQ], FP16, name="ktf0", tag="ktf0")
                        vf0 = wk.tile([128, SEQ], FP16, name="vf0", tag="vf0")
                        nc.sync.dma_start(
                            ktf0.rearrange("r (j m) -> r j m", j=N_CORES),
                            ago[:, :, 0:256].bitcast(FP16),
                        )
                        nc.scalar.dma_start(
                            vf0.rearrange("r (j d) -> r j d", j=N_CORES),
                            ago[:, :, 256:512].bitcast(FP16),
                        )
                        s_psA = psB.tile([128, 512], F32, name="s_psA",
                                         tag="small")
                        s_psB = psB.tile([128, 512], F32, name="s_psB",
                                         tag="small")
                        MM(s_psA[:], qt0[:], ktf0[:, 0:512])
                        MM(s_psB[:], qt0[:], ktf0[:, 512:1024])
                        rmA = wk.tile([128, 1], F32, name="rmA", tag="sc1")
                        rmB = wk.tile([128, 1], F32, name="rmB", tag="sc2")
                        nc.vector.reduce_max(rmA[:], s_psA[:], axis=AX.X)
                        nc.vector.reduce_max(rmB[:], s_psB[:], axis=AX.X)
                        rowmax = wk.tile([128, 1], F32, name="rowmax", tag="sc8")
                        nc.vector.tensor_tensor(rowmax[:], rmA[:], rmB[:],
                                                mybir.AluOpType.max)
                        negmax = wk.tile([128, 1], F32, name="negmax", tag="sc9")
                        nc.vector.tensor_scalar_mul(negmax[:], rowmax[:], -1.0)
                        rs0 = wk.tile([128, 1], F32, name="rs0", tag="sc1")
                        rs1 = wk.tile([128, 1], F32, name="rs1", tag="sc2")
                        p_sb = wk.tile([128, SEQ], FP16, name="p_sb0",
                                       tag="p_sb0")
                        tpp = psB.tile([128, SEQ], FP16, name="tpp",
                                       tag="tpp0", bufs=1)
                        pt0 = wk.tile([128, SEQ], FP16, name="pt0", tag="pt0")
                        at_ps = psB.tile([128, 512], F32, name="at_ps",
                                         tag="small")
                        nc.scalar.activation(
                            p_sb[:, 0:512], s_psA[:], AF.Exp, bias=negmax[:],
                            accum_out=rs0[:],
                        )
                        nc.scalar.activation(
                            p_sb[:, 512:1024], s_psB[:], AF.Exp, bias=negmax[:],
                            accum_out=rs1[:],
                        )
                        for j in range(8):
                            nc.tensor.transpose(
                                tpp[:, 128 * j : 128 * (j + 1)],
                                p_sb[:, 128 * j : 128 * (j + 1)],
                                ident_hsb[:],
                            )
                        nc.vector.tensor_copy(pt0[:, 0:512], tpp[:, 0:512])
                        nc.scalar.copy(pt0[:, 512:1024], tpp[:, 512:1024])
                        for j in range(8):
                            MM(
                                at_ps[:, 0:128],
                                vf0[:, 128 * j : 128 * (j + 1)],
                                pt0[:, 128 * j : 128 * (j + 1)],
                                start=(j == 0),
                                stop=(j == 7),
                            )
                        rowsum = wk.tile([128, 1], F32, name="rowsum", tag="sc3")
                        nc.vector.tensor_tensor(rowsum[:], rs0[:], rs1[:], ADD)
                        recip = wk.tile([128, 1], F32, name="recip", tag="sc4")
                        nc.vector.reciprocal(recip[:], rowsum[:])
                        at_sb = wk.tile([128, 128], FP16, name="at_sb",
                                        tag="at_sb")
                        nc.vector.tensor_copy(at_sb[:], at_ps[:, 0:128])
                    else:
                        # =================== steady-state block ===============
                        # pre-AG payload pack (uses qkv_raw, rin2 from blk-1)
                        ksc16 = wk.tile([128, 128], FP16, name="ksc16",
                                        tag="k16")
                        nc.vector.tensor_scalar_mul(ksc16[:],
                                                    qkv_raw[:, 128:256],
                                                    kscale[:])
                        tpk = psQ.tile([128, 1024], U8, name="tpk",
                                       tag="tpk")
                        nc.tensor.transpose(tpk[:, 0:256].bitcast(FP16),
                                            ksc16[:], ident_hsb[:])
                        kv_out = wk.tile([128, 256], U8, name="kv_out",
                                         tag="kvout")
                        nc.gpsimd.tensor_scalar_mul(
                            kv_out[:, 128:256].bitcast(FP8),
                            qkv_raw[:, 256:384], kscale[:])
                        nc.scalar.copy(kv_out[:, 0:128].bitcast(FP8),
                                       tpk[:, 0:256].bitcast(FP16))
                        ag_in = dram.tile([128, 256], U8, name="ag_in",
                                          tag="agi")
                        nc.sync.dma_start(ag_in[:, 128:256],
                                          kv_out[:, 128:256])
                        nc.sync.dma_start(ag_in[:, 0:128], kv_out[:, 0:128])
                        ag_out = dram.tile(
                            [N_CORES * 128, 256], U8, name="ag_out",
                            tag="ago", addr_space="Shared",
                        )
                        nc.gpsimd.collective_compute(
                            "AllGather", mybir.AluOpType.bypass,
                            replica_groups=rg,
                            ins=[ag_in.opt()], outs=[ag_out.opt()],
                        )

                        # ---- during AG: h, hT, P1/P2 for this block; Q^T ----
                        qs16 = wk.tile([128, 128], FP16, name="qs16",
                                       tag="q16")
                        nc.gpsimd.tensor_scalar_mul(qs16[:], qkv_raw[:, 0:128],
                                                    kscale[:])
                        nc.tensor.transpose(tpk[:, 256:512].bitcast(FP16),
                                            qs16[:], ident_hsb[:])
                        qt = wk.tile([128, 128], FP8, name="qt", tag="qt")
                        nc.vector.tensor_copy(qt[:],
                                              tpk[:, 256:512].bitcast(FP16))

                        h_sb = wk.tile([128, FEAT], F32R, name="h_sb",
                                       tag="h")
                        nc.scalar.activation(h_sb[:], X_sb[:], AF.Copy,
                                             scale=rin2[:])
                        tpb = psA.tile([128, 1024], F32R, name="tpb", tag="big")
                        for ft in range(NF):
                            nc.tensor.transpose(
                                tpb[:, 128 * ft : 128 * (ft + 1)],
                                h_sb[:, 128 * ft : 128 * (ft + 1)],
                                ident_r,
                            )
                        hT = wk.tile([128, FEAT], F32R, name="hT", tag="hT")
                        nc.vector.tensor_copy(hT[:, 0:384], tpb[:, 0:384])
                        nc.scalar.copy(hT[:, 384:768], tpb[:, 384:768])
                        g_ps = psA.tile([128, 1024], F32, name="g_ps",
                                        tag="big")
                        for ft in range(NF):
                            MM(
                                g_ps[:, 0:512],
                                hT[:, 128 * ft : 128 * (ft + 1)],
                                wm_sb[:, FEAT * ft : FEAT * ft + 512],
                                start=(ft == 0),
                                stop=(ft == NF - 1),
                            )
                            MM(
                                g_ps[:, 512:768],
                                hT[:, 128 * ft : 128 * (ft + 1)],
                                wm_sb[:, FEAT * ft + 512 : FEAT * (ft + 1)],
                                start=(ft == 0),
                                stop=(ft == NF - 1),
                            )
                        P1 = wk.tile([128, FEAT], F32, name="P1", tag="P1")
                        nc.vector.tensor_copy(P1[:, 0:384], g_ps[:, 0:384])
                        nc.scalar.copy(P1[:, 384:768], g_ps[:, 384:768])
                        if not last:
                            p2_ps = psB.tile([128, 512], F32, name="p2_ps",
                                             tag="small")
                            for ft in range(NF):
                                MM(
                                    p2_ps[:, 0:384],
                                    hT[:, 128 * ft : 128 * (ft + 1)],
                                    wmqkv_sb[:, 384 * ft : 384 * (ft + 1)],
                                    start=(ft == 0),
                                    stop=(ft == NF - 1),
                                )
                            P2s = wk.tile([128, 384], F32, name="P2s",
                                          tag="P2s")
                            nc.scalar.copy(P2s[:], p2_ps[:, 0:384])

                        # ---- post-AG: m-major fp8 attention (no P^T
                        # transposes: scores computed as S^T chunks, exp
                        # writes P^T to SBUF directly; Z via pt_j^T @ ones) --
                        ago = ag_out.rearrange("(j r) c -> r j c", r=128)
                        ktf = wk.tile([128, SEQ], FP8, name="ktf", tag="ktf")
                        vf = wk.tile([128, SEQ], FP8, name="vf", tag="vf")
                        ktf_r = ktf.rearrange("r (j m) -> r j m", j=N_CORES)
                        nc.sync.dma_start(ktf_r[:, 0:4, :],
                                          ago[:, 0:4, 0:128].bitcast(FP8))
                        nc.sync.dma_start(ktf_r[:, 4:8, :],
                                          ago[:, 4:8, 0:128].bitcast(FP8))
                        nc.sync.dma_start(
                            vf.rearrange("r (j d) -> r j d", j=N_CORES),
                            ago[:, :, 128:256].bitcast(FP8),
                        )
                        vf16 = wk.tile([128, SEQ], FP16, name="vf16",
                                       tag="vf16")
                        nc.gpsimd.tensor_copy(vf16[:], vf[:])
                        s_ps = psA.tile([128, 1024], F32, name="s_ps",
                                        tag="big")
                        for j in range(8):
                            MM(s_ps[:, 128 * j : 128 * (j + 1)],
                               ktf[:, 128 * j : 128 * (j + 1)], qt[:])
                        pt = wk.tile([128, SEQ], FP16, name="pt", tag="pt")
                        nc.scalar.activation(
                            pt[:, 0:512], s_ps[:, 0:512], AF.Exp,
                            scale=1.0 / (KSC * KSC),
                        )
                        nc.scalar.activation(
                            pt[:, 512:1024], s_ps[:, 512:1024], AF.Exp,
                            scale=1.0 / (KSC * KSC),
                        )
                        at_ps = psB.tile([128, 512], F32, name="at_ps",
                                         tag="small")
                        for j in range(8):
                            MM(
                                at_ps[:, 0:128],
                                vf16[:, 128 * j : 128 * (j + 1)],
                                pt[:, 128 * j : 128 * (j + 1)],
                                start=(j == 0),
                                stop=(j == 7),
                            )
                            MM(
                                tpk[:, 512:516].bitcast(F32),
                                pt[:, 128 * j : 128 * (j + 1)],
                                ones_f8[:],
                                start=(j == 0),
                                stop=(j == 7),
                            )
                        recip = wk.tile([128, 1], F32, name="recip", tag="sc4")
                        nc.vector.reciprocal(recip[:],
                                             tpk[:, 512:516].bitcast(F32))
                        at_sb = wk.tile([128, 128], FP16, name="at_sb",
                                        tag="at_sb")
                        nc.vector.tensor_copy(at_sb[:], at_ps[:, 0:128])

                    # ============ shared X / qkv_raw / rin2 update ============
                    # q2 MM first so its sem lands earliest (DVE unparks the
                    # most-recently-ready wait: X halves then win over qkv).
                    if not last:
                        q2_ps = psB.tile([128, 512], F32, name="q2_ps",
                                         tag="small")
                        MM(q2_ps[:, 0:384], at_sb[:], wowmq_sb[:])
                    x_ps = psA.tile([128, 1024], F32, name="x_ps", tag="big")
                    MM(x_ps[:, 0:512], at_sb[:], wowm_sb[:, 0:512])
                    MM(x_ps[:, 512:768], at_sb[:], wowm_sb[:, 512:768])
                    if not last:
                        qkv_raw = wk.tile([128, 384], F32, name="qkv_raw",
                                          tag="qraw")
                        nc.vector.scalar_tensor_tensor(
                            qkv_raw[:], q2_ps[:, 0:384], recip[:], P2s[:],
                            op0=MULT, op1=ADD,
                        )
                    X_sb = wk.tile([128, FEAT], F32, name="X_sb", tag="X")
                    nc.vector.scalar_tensor_tensor(
                        X_sb[:, 0:384], x_ps[:, 0:384], recip[:],
                        P1[:, 0:384], op0=MULT, op1=ADD,
                    )
                    nc.vector.scalar_tensor_tensor(
                        X_sb[:, 384:768], x_ps[:, 384:768], recip[:],
                        P1[:, 384:768], op0=MULT, op1=ADD,
                    )
                    sq2 = wk.tile([128, FEAT], F32, name="sq2", tag="sq")
                    ssa = wk.tile([128, 1], F32, name="ssa", tag="sc5")
                    ssb = wk.tile([128, 1], F32, name="ssb", tag="sc5b")
                    nc.scalar.activation(sq2[:, 0:384], X_sb[:, 0:384],
                                         AF.Square, accum_out=ssa[:])
                    nc.scalar.activation(sq2[:, 384:768], X_sb[:, 384:768],
                                         AF.Square, accum_out=ssb[:])
                    ss2 = wk.tile([128, 1], F32, name="ss2", tag="sc5c")
                    nc.vector.tensor_tensor(ss2[:], ssa[:], ssb[:], ADD)
                    nrm2 = wk.tile([128, 1], F32, name="nrm2", tag="sc6")
                    nc.scalar.activation(nrm2[:], ss2[:], AF.Sqrt)
                    nrm2c = wk.tile([128, 1], F32, name="nrm2c", tag="sc6b")
                    nc.vector.tensor_scalar_max(nrm2c[:], nrm2[:], 1e-12)
                    rin2 = wk.tile([128, 1], F32, name="rin2", tag="sc7")
                    nc.vector.reciprocal(rin2[:], nrm2c[:])
                    if not last:
                        kscale = wk.tile([128, 1], F32, name="kscale",
                                         tag="sc9")
                        nc.vector.tensor_scalar_mul(kscale[:], rin2[:], KSC)

                # ---- final h^T (fp16), AllGathered to all cores ----
                h_sb = wk.tile([128, FEAT], F32, name="h_sbf", tag="h")
                nc.scalar.activation(h_sb[:], X_sb[:], AF.Copy, scale=rin2[:])
                tpf = psA.tile([128, 1024], F32, name="tpf", tag="big")
                for ft in range(NF):
                    nc.tensor.transpose(
                        tpf[:, 128 * ft : 128 * (ft + 1)],
                        h_sb[:, 128 * ft : 128 * (ft + 1)],
                        ident_sb[:],
                    )
                hTf = wk.tile([128, FEAT], FP16, name="hTf", tag="hTf")
                nc.vector.tensor_copy(hTf[:, 0:384], tpf[:, 0:384])
                nc.scalar.copy(hTf[:, 384:768], tpf[:, 384:768])
                agh_in = dram.tile([FEAT, 128], FP16, bufs=1)
                nc.sync.dma_start(
                    agh_in.rearrange("(t p) m -> p t m", p=128),
                    hTf.rearrange("p (t m) -> p t m", t=NF),
                )
                agh_out = dram.tile(
                    [N_CORES * FEAT, 128], FP16, addr_space="Shared", bufs=1
                )
                nc.gpsimd.collective_compute(
                    "AllGather", mybir.AluOpType.bypass, replica_groups=rg,
                    ins=[agh_in.opt()], outs=[agh_out.opt()],
                )

            # ======================= out-projection ==========================
            with tc.tile_pool(name="oph", bufs=2) as op:
                htf_sb = op.tile([128, NF * SEQ], FP16, name="htf_sb", tag="htf",
                                 bufs=1)
                agh_r = agh_out.rearrange("(j t p) m -> p t j m", t=NF, p=128)
                for ft in range(NF):
                    nc.sync.dma_start(
                        htf_sb[:, SEQ * ft : SEQ * (ft + 1)].rearrange(
                            "p (j m) -> p j m", j=N_CORES
                        ),
                        agh_r[:, ft, :, :],
                    )

                wout_r = wout_h.rearrange("(t p) v -> p t v", p=128)
                for c in range(NVT):
                    woc = op.tile([128, NF * 128], FP16, name="woc", tag="woc",
                                  bufs=3)
                    # gpsimd queue: prefetch runs during the h AllGather
                    nc.gpsimd.dma_start(
                        woc.rearrange("p (t v) -> p t v", t=NF),
                        wout_r[:, :, 128 * c : 128 * (c + 1)],
                    )
                    L_ps = psA.tile([128, 1024], F32, name="L_ps", tag="big")
                    for ft in range(NF):
                        MM(
                            L_ps[:, 0:512],
                            woc[:, 128 * ft : 128 * (ft + 1)],
                            htf_sb[:, SEQ * ft : SEQ * ft + 512],
                            start=(ft == 0),
                            stop=(ft == NF - 1),
                        )
                        MM(
                            L_ps[:, 512:1024],
                            woc[:, 128 * ft : 128 * (ft + 1)],
                            htf_sb[:, SEQ * ft + 512 : SEQ * (ft + 1)],
                            start=(ft == 0),
                            stop=(ft == NF - 1),
                        )
                    l_sb = op.tile([128, SEQ], FP16, name="l_sb", tag="l_sb")
                    nc.scalar.copy(l_sb[:, 0:512], L_ps[:, 0:512])
                    nc.scalar.copy(l_sb[:, 512:1024], L_ps[:, 512:1024])

                    nc.vector.max(topv_all[:, RW * c : RW * c + 8], l_sb[:])
                    prev = l_sb
                    for r in range(1, rounds):
                        mrb = op.tile(
                            [128, SEQ], FP16, name="mrb", tag=f"mrb{r % 2}"
                        )
                        nc.vector.match_replace(
                            mrb[:],
                            topv_all[:, RW * c + 8 * (r - 1) : RW * c + 8 * r],
                            prev[:],
                            -60000.0,
                        )
                        nc.vector.max(
                            topv_all[:, RW * c + 8 * r : RW * c + 8 * (r + 1)],
                            mrb[:],
                        )
                        prev = mrb

                nc.sync.dma_start(
                    topv.rearrange("(c p) w -> p c w", p=128),
                    topv_all.rearrange("p (c w) -> p c w", c=NVT),
                )

    _split_excess_waits(nc)
    return nc


_CACHE = {}


def _get_program(nblocks, rounds):
    key = ("fast2", nblocks, rounds)
    if key not in _CACHE:
        _CACHE[key] = _build_fast2(nblocks, rounds)
    return _CACHE[key]


def kernel(x, pe, W_emb, b_emb, Wq, bq, Wk, bk, Wv, bv, Wo, bo, W1, b1, Wout,
           bout, k, _profile=False, _nblocks=NBLOCKS):
    x = np.asarray(x, dtype=np.float32).reshape(SEQ, VOCAB)
    pe = np.asarray(pe, dtype=np.float32)
    W_emb = np.asarray(W_emb, dtype=np.float32)
    Wq = np.asarray(Wq, dtype=np.float32)
    Wk = np.asarray(Wk, dtype=np.float32)
    Wv = np.asarray(Wv, dtype=np.float32)
    Wo = np.asarray(Wo, dtype=np.float32)
    W1 = np.asarray(W1, dtype=np.float32)
    Wout = np.asarray(Wout, dtype=np.float32)
    b_emb = np.asarray(b_emb, dtype=np.float32)
    bq = np.asarray(bq, dtype=np.float32)
    bk = np.asarray(bk, dtype=np.float32)
    bv = np.asarray(bv, dtype=np.float32)
    bo = np.asarray(bo, dtype=np.float32)
    b1 = np.asarray(b1, dtype=np.float32)
    bout = np.asarray(bout, dtype=np.float32)
    k = int(np.asarray(k))
    rounds = max(1, math.ceil(k / 8))
    assert rounds * 8 <= 24, f"k={k} too large for this kernel"
    assert not (np.any(bq) or np.any(bk) or np.any(bv) or np.any(bo)
                or np.any(b1) or np.any(bout)), "bias path not supported"

    nc = _get_program(_nblocks, rounds)

    # host-side shard prep
    VTOT = N_CORES * VP
    wemb_pad = np.zeros((VTOT, FEAT), dtype=np.float32)
    wemb_pad[:VOCAB, :] = W_emb
    wout_pad = np.zeros((FEAT, VTOT), dtype=np.float32)
    wout_pad[:, :VOCAB] = Wout
    wqkv = np.ascontiguousarray(np.concatenate([Wq, Wk, Wv], axis=1))
    ident = np.eye(128, dtype=np.float32)

    xT_pad = np.zeros((VTOT, SEQ), dtype=np.float32)
    xT_pad[:VOCAB, :] = x.T
    W1_64 = W1.astype(np.float64)
    Wm64 = W1_64 + W1_64 @ W1_64
    Wm = Wm64.astype(np.float32)
    Wmqkv64 = Wm64 @ wqkv.astype(np.float64)
    Wmqkv = Wmqkv64.astype(np.float32)
    Wo64 = Wo.astype(np.float64)
    WoWm = (Wo64 @ Wm64).astype(np.float32)
    WoWmqkv = (Wo64 @ Wmqkv64).astype(np.float32)
    ident_b = ident.astype(np.float16)

    in_maps = []
    for i in range(N_CORES):
        m = {
            "xT_h": np.ascontiguousarray(
                xT_pad[VP * i : VP * (i + 1), :]
            ).astype(np.float16),
            "wemb_h": np.ascontiguousarray(
                wemb_pad[VP * i : VP * (i + 1), :]
            ).astype(np.float16),
            "wqkv": wqkv,
            "wm": Wm,
            "wmqkv": Wmqkv,
            "wowm_h": WoWm.astype(np.float16),
            "wowmq_h": WoWmqkv.astype(np.float16),
            "wout_h": np.ascontiguousarray(
                wout_pad[:, VP * i : VP * (i + 1)]
            ).astype(np.float16),
            "pe_i": np.ascontiguousarray(
                pe[SSH * i : SSH * (i + 1), :] + b_emb
            ),
            "ident": ident,
            "ident_b": ident_b,
        }
        in_maps.append(m)

    res = None
    for attempt in range(3):
        try:
            res = run_bass_kernel_spmd(
                nc, in_maps, core_ids=list(range(N_CORES)), trace=_profile
            )
            break
        except Exception:
            # transient NRT/axon failures (e.g. NRT_EXEC_UNIT_UNRECOVERABLE)
            # have been observed; retry with the cached executable
            if attempt == 2:
                raise
            import time as _time
            _time.sleep(5)

    RW = 8 * rounds
    full = np.concatenate(
        [np.asarray(res.results[i]["topv"], dtype=np.float32).reshape(VP, RW)
         for i in range(N_CORES)], axis=0
    )
    vals = full[:VOCAB, :k]  # [VOCAB, k]
    out = np.ascontiguousarray(vals.T)[None, :, :]  # [1, k, VOCAB]

    if _profile:
        return out.astype(np.float32), res
    return out.astype(np.float32)


# revision 23
# speedup vs baseline: 1.2149x; 1.0173x over previous
"""Trainium2 Bass kernel for nn_GPT3_56934086476265.

96-block GPT-style transformer, B=1, N=1024, FEAT=768, ATTN=128, VOCAB=32000.

Sharding (8 cores, 1 chip):
  - Embedding (x @ W_emb): vocab-contraction sharded; ReduceScatter hands each
    core its 128-row sequence shard.
  - 96 blocks: sequence-parallel (128 seq rows per core). Per block one
    AllGather exchanges K^T|V (both fp8, 128x256B per rank).
  - Out-projection + top-k: hidden state AllGathered once; each core computes
    logits^T for its 4096 vocab columns and takes top-k along the sequence
    axis with max8 + match_replace + max8.

Critical-path restructure (vs the earlier version): with all biases zero the
per-block recurrence
    n1pre_t = h_t + A~_t @ Wo,  X_t = n1pre_t @ Wm,  h_{t+1} = X_t / |X_t|
is evaluated as
    X_t        = P1 + (A~_t @ WoWm) / Z
    qkv_raw    = P2 + (A~_t @ WoWmWqkv) / Z         (payload for block t+1)
where P1 = h_t @ Wm and P2 = h_t @ WmWqkv are computed DURING the AllGather
window (PE is otherwise idle there), WoWm / WoWmWqkv are host-precomputed
rank-128 factors, and 1/Z (softmax normalizer) folds into the PSUM->SBUF
scalar_tensor_tensor. Only the attention + rank-128 matmuls + row-norm +
payload pack remain on the exposed post-AllGather chain.
"""

import math

import numpy as np

import concourse.bass as bass
import concourse.mybir as mybir
import concourse.tile as tile
from concourse.bass_utils import run_bass_kernel_spmd

N_CORES = 8
SEQ = 1024
FEAT = 768
ATTN = 128
NBLOCKS = 96
VOCAB = 32000
VP = 4096          # padded vocab per core (8*4096 = 32768 >= 32000)
SSH = 128          # sequence rows per core
NF = FEAT // 128   # 6 feature tiles
NVT = VP // 128    # 32 vocab tiles per core

dt = mybir.dt
F32 = dt.float32
F32R = dt.float32r
BF16 = dt.bfloat16
FP16 = dt.float16
FP8 = dt.float8e4
U8 = dt.uint8
KSC = 256.0
ADD = mybir.AluOpType.add
MULT = mybir.AluOpType.mult
AF = mybir.ActivationFunctionType
AX = mybir.AxisListType

_WAITFIX_UID = [0]


def _split_excess_waits(nc, max_keep=1):
    """walrus codegen on this toolchain only encodes one attached sync-wait on
    several instruction formats (fp32 Matmult lowers to LDWEIGHTS with a single
    wait slot; Drain/NoOp similar). Move excess waits onto standalone
    EventSemaphore instructions just before each over-budget instruction."""
    n = 0
    for f in nc.m.functions:
        for b in f.blocks:
            insts = list(b.instructions)
            out = []
            changed = False
            for ins in insts:
                si = ins.sync_info
                if si is not None and si.on_wait and len(si.on_wait) > max_keep:
                    waits = list(si.on_wait)
                    excess, keep = waits[:-max_keep], waits[-max_keep:]
                    for w in excess:
                        _WAITFIX_UID[0] += 1
                        es = mybir.InstEventSemaphore(
                            name=f"I-waitfix-{_WAITFIX_UID[0]}", ins=[], outs=[]
                        )
                        es.engine = ins.engine
                        es.sync_info = mybir.SyncInfo(on_wait=[w], on_update=[])
                        out.append(es)
                        n += 1
                    ins.sync_info = mybir.SyncInfo(
                        on_wait=keep, on_update=si.on_update
                    )
                    changed = True
                out.append(ins)
            if changed:
                b.instructions = out
    return n


def _build_fast2(nblocks, rounds):
    nc = bass.Bass(num_devices=N_CORES)

    xT_h = nc.declare_dram_parameter("xT_h", [VP, SEQ], FP16, isOutput=False)
    wemb_h = nc.declare_dram_parameter("wemb_h", [VP, FEAT], FP16, isOutput=False)
    wqkv = nc.declare_dram_parameter("wqkv", [FEAT, 3 * ATTN], F32, isOutput=False)
    wm = nc.declare_dram_parameter("wm", [FEAT, FEAT], F32, isOutput=False)
    wmqkv = nc.declare_dram_parameter("wmqkv", [FEAT, 3 * ATTN], F32,
                                      isOutput=False)
    wowm_h = nc.declare_dram_parameter("wowm_h", [ATTN, FEAT], FP16,
                                       isOutput=False)
    wowmq_h = nc.declare_dram_parameter("wowmq_h", [ATTN, 3 * ATTN], FP16,
                                        isOutput=False)
    wout_h = nc.declare_dram_parameter("wout_h", [FEAT, VP], FP16,
                                       isOutput=False)
    pe_i = nc.declare_dram_parameter("pe_i", [SSH, FEAT], F32, isOutput=False)
    ident = nc.declare_dram_parameter("ident", [128, 128], F32, isOutput=False)
    ident_b = nc.declare_dram_parameter("ident_b", [128, 128], FP16,
                                        isOutput=False)

    RW = 8 * rounds
    topv = nc.declare_dram_parameter("topv", [VP, RW], FP16, isOutput=True)

    rg = [list(range(N_CORES))]
    fr = lambda ap: ap.bitcast(F32R)

    with tile.TileContext(nc) as tc:
        with (
            tc.tile_pool(name="const", bufs=1) as cpool,
            tc.tile_pool(name="psA", bufs=2, space="PSUM") as psA,
            tc.tile_pool(name="psB", bufs=2, space="PSUM") as psB,
            tc.tile_pool(name="psQ", bufs=1, space="PSUM") as psQ,
            tc.tile_pool(name="dram", bufs=2, space="DRAM") as dram,
        ):
            # ---- resident constants ----
            ident_sb = cpool.tile([128, 128], F32)
            nc.sync.dma_start(ident_sb[:], ident[:])
            ident_rsb = cpool.tile([128, 128], F32R)
            nc.sync.dma_start(ident_rsb[:], fr(ident[:]))
            ident_r = ident_rsb[:]
            ident_hsb = cpool.tile([128, 128], FP16)
            nc.sync.dma_start(ident_hsb[:], ident_b[:])
            ones_f8 = cpool.tile([128, 1], FP16)
            nc.vector.memset(ones_f8[:], KSC)
            pe_sb = cpool.tile([128, FEAT], F32)
            nc.sync.dma_start(pe_sb[:], pe_i[:])
            wqkv_sb = cpool.tile([128, NF * 384], F32R)
            wm_sb = cpool.tile([128, NF * FEAT], F32R)
            wmqkv_sb = cpool.tile([128, NF * 384], F32R)
            wowm_sb = cpool.tile([128, FEAT], FP16)
            wowmq_sb = cpool.tile([128, 384], FP16)
            topv_all = cpool.tile([128, NVT * RW], FP16)

            MM = nc.tensor.matmul

            # =========================== embedding ===========================
            rs_in = dram.tile([SEQ, FEAT], F32, bufs=1)
            rs_outA = dram.tile([SSH // 2, FEAT], F32, bufs=1)
            rs_outB = dram.tile([SSH // 2, FEAT], F32, bufs=1)
            h0_sb = cpool.tile([128, FEAT], F32, name="h0_sb")

            with tc.tile_pool(name="embw", bufs=1) as embw:
                wemb_sb = embw.tile([128, NVT * FEAT], FP16)
                wr = wemb_h.rearrange("(c p) f -> p c f", p=128)
                wsb = wemb_sb.rearrange("p (c f) -> p c f", c=NVT)
                for q in range(4):
                    eng = nc.sync if q % 2 == 0 else nc.scalar
                    eng.dma_start(
                        wsb[:, 8 * q : 8 * (q + 1), :], wr[:, 8 * q : 8 * (q + 1), :]
                    )
                # whole x^T resident in two fp16 tiles (16 vocab chunks each)
                xr = xT_h.rearrange("(c p) s -> p c s", p=128)
                xa = embw.tile([128, 16 * SEQ], FP16)
                xb = embw.tile([128, 16 * SEQ], FP16)
                xav = xa.rearrange("p (c s) -> p c s", c=16)
                xbv = xb.rearrange("p (c s) -> p c s", c=16)
                for q in range(4):
                    nc.sync.dma_start(
                        xav[:, 4 * q : 4 * (q + 1), :],
                        xr[:, 4 * q : 4 * (q + 1), :],
                    )
                    nc.scalar.dma_start(
                        xbv[:, 4 * q : 4 * (q + 1), :],
                        xr[:, 16 + 4 * q : 16 + 4 * (q + 1), :],
                    )
                # block-phase constants arrive behind the embedding data
                nc.sync.dma_start(
                    wqkv_sb.rearrange("p (t d) -> p t d", t=NF),
                    fr(wqkv.rearrange("(t p) d -> p t d", p=128)),
                )
                nc.sync.dma_start(
                    wmqkv_sb.rearrange("p (t d) -> p t d", t=NF),
                    fr(wmqkv.rearrange("(t p) d -> p t d", p=128)),
                )
                nc.scalar.dma_start(
                    wm_sb.rearrange("p (t d) -> p t d", t=NF),
                    fr(wm.rearrange("(t p) d -> p t d", p=128)),
                )
                nc.scalar.dma_start(wowm_sb[:], wowm_h[:])
                nc.scalar.dma_start(wowmq_sb[:], wowmq_h[:])
                for t in range(SEQ // 128):
                    hp = psA.tile([128, 1024], F32, name="hp", tag="big")
                    for c in range(NVT):
                        src = xa if c < 16 else xb
                        lhs = src[:, SEQ * (c % 16) + 128 * t :
                                  SEQ * (c % 16) + 128 * (t + 1)]
                        MM(
                            hp[:, 0:512],
                            lhs,
                            wemb_sb[:, FEAT * c : FEAT * c + 512],
                            start=(c == 0),
                            stop=(c == NVT - 1),
                        )
                        MM(
                            hp[:, 512:768],
                            lhs,
                            wemb_sb[:, FEAT * c + 512 : FEAT * (c + 1)],
                            start=(c == 0),
                            stop=(c == NVT - 1),
                        )
                    hp_sb = cpool.tile([128, FEAT], F32, name="hp_sb",
                                       tag="hp_sb", bufs=2)
                    if t % 2 == 0:
                        nc.vector.tensor_copy(hp_sb[:], hp[:, 0:FEAT])
                    else:
                        nc.scalar.copy(hp_sb[:], hp[:, 0:FEAT])
                    nc.sync.dma_start(rs_in[128 * t : 128 * (t + 1), :], hp_sb[:])
                    if t == 3:
                        # first-half ReduceScatter hides under tiles 4-7
                        nc.gpsimd.collective_compute(
                            "ReduceScatter", ADD, replica_groups=rg,
                            ins=[rs_in[0:512, :].opt()], outs=[rs_outA.opt()],
                        )

                nc.gpsimd.collective_compute(
                    "ReduceScatter", ADD, replica_groups=rg,
                    ins=[rs_in[512:1024, :].opt()], outs=[rs_outB.opt()],
                )
                h0_tmp = cpool.tile([128, FEAT], F32, name="h0_tmp", tag="hp_sb",
                                    bufs=2)
                nc.sync.dma_start(h0_tmp[0:64, :], rs_outA[:])
                nc.sync.dma_start(h0_tmp[64:128, :], rs_outB[:])
                nc.vector.tensor_tensor(h0_sb[:], h0_tmp[:], pe_sb[:], ADD)

            # =========================== blocks ==============================
            with tc.tile_pool(name="blk", bufs=2) as wk:
                at_sb = None
                recip = None
                P1 = None
                P2s = None
                rin2 = None
                X_sb = None
                qkv_raw = None
                qt = None
                kscale = None

                for blk in range(nblocks):
                    last = blk == nblocks - 1
                    if blk == 0:
                        # ---- bootstrap: qkv0 = h0 @ Wqkv, fp16 payload ----
                        tpb = psA.tile([128, 1024], F32, name="tpb0", tag="big")
                        for ft in range(NF):
                            nc.tensor.transpose(
                                tpb[:, 128 * ft : 128 * (ft + 1)],
                                h0_sb[:, 128 * ft : 128 * (ft + 1)],
                                ident_sb[:],
                            )
                        hT = wk.tile([128, FEAT], F32R, name="hT", tag="hT")
                        nc.vector.tensor_copy(hT[:, 0:384], tpb[:, 0:384])
                        nc.scalar.copy(hT[:, 384:768], tpb[:, 384:768])
                        q_ps = psB.tile([128, 512], F32, name="q_ps", tag="small")
                        for ft in range(NF):
                            MM(
                                q_ps[:, 0:384],
                                hT[:, 128 * ft : 128 * (ft + 1)],
                                wqkv_sb[:, 384 * ft : 384 * (ft + 1)],
                                start=(ft == 0),
                                stop=(ft == NF - 1),
                            )
                        qkv_sb = wk.tile([128, 384], F32, name="qkv_sb",
                                         tag="qkv_sb")
                        nc.vector.tensor_copy(qkv_sb[:], q_ps[:, 0:384])
                        tpk0 = psB.tile([128, 512], F32, name="tpk0",
                                        tag="small")
                        nc.tensor.transpose(tpk0[:, 0:128], qkv_sb[:, 128:256],
                                            ident_sb[:])
                        kv_out0 = wk.tile([128, 512], U8, name="kv_out0",
                                          tag="kv0")
                        nc.vector.tensor_copy(
                            kv_out0[:, 0:256].bitcast(FP16), tpk0[:, 0:128])
                        nc.scalar.copy(
                            kv_out0[:, 256:512].bitcast(FP16),
                            qkv_sb[:, 256:384])
                        ag_in = dram.tile([128, 512], U8, name="ag_in0",
                                          tag="agi0")
                        nc.sync.dma_start(ag_in[:], kv_out0[:])
                        ag_out = dram.tile(
                            [N_CORES * 128, 512], U8, name="ag_out0",
                            tag="ago0", addr_space="Shared",
                        )
                        nc.gpsimd.collective_compute(
                            "AllGather", mybir.AluOpType.bypass,
                            replica_groups=rg,
                            ins=[ag_in.opt()], outs=[ag_out.opt()],
                        )
                        # ---- during AG0: Q^T, P1/P2 from h0 ----
                        nc.tensor.transpose(tpk0[:, 128:256], qkv_sb[:, 0:128],
                                            ident_sb[:])
                        qt0 = wk.tile([128, 128], FP16, name="qt0", tag="qt0")
                        nc.vector.tensor_copy(qt0[:], tpk0[:, 128:256])
                        g_ps = psA.tile([128, 1024], F32, name="g_ps", tag="big")
                        for ft in range(NF):
                            MM(
                                g_ps[:, 0:512],
                                hT[:, 128 * ft : 128 * (ft + 1)],
                                wm_sb[:, FEAT * ft : FEAT * ft + 512],
                                start=(ft == 0),
                                stop=(ft == NF - 1),
                            )
                            MM(
                                g_ps[:, 512:768],
                                hT[:, 128 * ft : 128 * (ft + 1)],
                                wm_sb[:, FEAT * ft + 512 : FEAT * (ft + 1)],
                                start=(ft == 0),
                                stop=(ft == NF - 1),
                            )
                        p2_ps = psB.tile([128, 512], F32, name="p2_ps",
                                         tag="small")
                        for ft in range(NF):
                            MM(
                                p2_ps[:, 0:384],
                                hT[:, 128 * ft : 128 * (ft + 1)],
                                wmqkv_sb[:, 384 * ft : 384 * (ft + 1)],
                                start=(ft == 0),
                                stop=(ft == NF - 1),
                            )
                        P1 = wk.tile([128, FEAT], F32, name="P1", tag="P1")
                        nc.vector.tensor_copy(P1[:, 0:384], g_ps[:, 0:384])
                        nc.scalar.copy(P1[:, 384:768], g_ps[:, 384:768])
                        P2s = wk.tile([128, 384], F32, name="P2s", tag="P2s")
                        nc.scalar.copy(P2s[:], p2_ps[:, 0:384])

                        # ---- post-AG0: fp16 attention with max-subtract ----
                        ago = ag_out.rearrange("(j r) c -> r j c", r=128)
                        ktf0 = wk.tile([128, SEQ], FP16, name="ktf0", tag="ktf0")
                        vf0 = wk.tile([128, SEQ], FP16, name="vf0", tag="vf0")
                        nc.sync.dma_start(
                            ktf0.rearrange("r (j m) -> r j m", j=N_CORES),
                            ago[:, :, 0:256].bitcast(FP16),
                        )
                        nc.scalar.dma_start(
                            vf0.rearrange("r (j d) -> r j d", j=N_CORES),
                            ago[:, :, 256:512].bitcast(FP16),
                        )
                        s_psA = psB.tile([128, 512], F32, name="s_psA",
                                         tag="small")
                        s_psB = psB.tile([128, 512], F32, name="s_psB",
                                         tag="small")
                        MM(s_psA[:], qt0[:], ktf0[:, 0:512])
                        MM(s_psB[:], qt0[:], ktf0[:, 512:1024])
                        rmA = wk.tile([128, 1], F32, name="rmA", tag="sc1")
                        rmB = wk.tile([128, 1], F32, name="rmB", tag="sc2")
                        nc.vector.reduce_max(rmA[:], s_psA[:], axis=AX.X)
                        nc.vector.reduce_max(rmB[:], s_psB[:], axis=AX.X)
                        rowmax = wk.tile([128, 1], F32, name="rowmax", tag="sc8")
                        nc.vector.tensor_tensor(rowmax[:], rmA[:], rmB[:],
                                                mybir.AluOpType.max)
                        negmax = wk.tile([128, 1], F32, name="negmax", tag="sc9")
                        nc.vector.tensor_scalar_mul(negmax[:], rowmax[:], -1.0)
                        rs0 = wk.tile([128, 1], F32, name="rs0", tag="sc1")
                        rs1 = wk.tile([128, 1], F32, name="rs1", tag="sc2")
                        p_sb = wk.tile([128, SEQ], FP16, name="p_sb0",
                                       tag="p_sb0")
                        tpp = psB.tile([128, SEQ], FP16, name="tpp",
                                       tag="tpp0", bufs=1)
                        pt0 = wk.tile([128, SEQ], FP16, name="pt0", tag="pt0")
                        at_ps = psB.tile([128, 512], F32, name="at_ps",
                                         tag="small")
                        nc.scalar.activation(
                            p_sb[:, 0:512], s_psA[:], AF.Exp, bias=negmax[:],
                            accum_out=rs0[:],
                        )
                        nc.scalar.activation(
                            p_sb[:, 512:1024], s_psB[:], AF.Exp, bias=negmax[:],
                            accum_out=rs1[:],
                        )
                        for j in range(8):
                            nc.tensor.transpose(
                                tpp[:, 128 * j : 128 * (j + 1)],
                                p_sb[:, 128 * j : 128 * (j + 1)],
                                ident_hsb[:],
                            )
                        nc.vector.tensor_copy(pt0[:, 0:512], tpp[:, 0:512])
                        nc.scalar.copy(pt0[:, 512:1024], tpp[:, 512:1024])
                        for j in range(8):
                            MM(
                                at_ps[:, 0:128],
                                vf0[:, 128 * j : 128 * (j + 1)],
                                pt0[:, 128 * j : 128 * (j + 1)],
                                start=(j == 0),
                                stop=(j == 7),
                            )
                        rowsum = wk.tile([128, 1], F32, name="rowsum", tag="sc3")
                        nc.vector.tensor_tensor(rowsum[:], rs0[:], rs1[:], ADD)
                        recip = wk.tile([128, 1], F32, name="recip", tag="sc4")
                        nc.vector.reciprocal(recip[:], rowsum[:])
                        at_sb = wk.tile([128, 128], FP16, name="at_sb",
                                        tag="at_sb")
                        nc.vector.tensor_copy(at_sb[:], at_ps[:, 0:128])
                    else:
                        # =================== steady-state block ===============
                        # pre-AG payload pack (uses qkv_raw, rin2 from blk-1)
                        ksc16 = wk.tile([128, 128], FP16, name="ksc16",
                                        tag="k16")
                        nc.vector.tensor_scalar_mul(ksc16[:],
                                                    qkv_raw[:, 128:256],
                                                    kscale[:])
                        tpk = psQ.tile([128, 1024], U8, name="tpk",
                                       tag="tpk")
                        nc.tensor.transpose(tpk[:, 0:256].bitcast(FP16),
                                            ksc16[:], ident_hsb[:])
                        kv_out = wk.tile([128, 256], U8, name="kv_out",
                                         tag="kvout")
                        nc.gpsimd.tensor_scalar_mul(
                            kv_out[:, 128:256].bitcast(FP8),
                            qkv_raw[:, 256:384], kscale[:])
                        nc.vector.tensor_copy(kv_out[:, 0:128].bitcast(FP8),
                                               tpk[:, 0:256].bitcast(FP16))
                        ag_in = dram.tile([128, 256], U8, name="ag_in",
                                          tag="agi")
                        nc.sync.dma_start(ag_in[:, 128:256],
                                          kv_out[:, 128:256])
                        nc.sync.dma_start(ag_in[:, 0:128], kv_out[:, 0:128])
                        ag_out = dram.tile(
                            [N_CORES * 128, 256], U8, name="ag_out",
                            tag="ago", addr_space="Shared",
                        )
                        nc.gpsimd.collective_compute(
                            "AllGather", mybir.AluOpType.bypass,
                            replica_groups=rg,
                            ins=[ag_in.opt()], outs=[ag_out.opt()],
                        )

                        # ---- during AG: h, hT, P1/P2 for this block; Q^T ----
                        qs16 = wk.tile([128, 128], FP16, name="qs16",
                                       tag="q16")
                        nc.gpsimd.tensor_scalar_mul(qs16[:], qkv_raw[:, 0:128],
                                                    kscale[:])
                        nc.tensor.transpose(tpk[:, 256:512].bitcast(FP16),
                                            qs16[:], ident_hsb[:])
                        qt = wk.tile([128, 128], FP8, name="qt", tag="qt")
                        nc.vector.tensor_copy(qt[:],
                                              tpk[:, 256:512].bitcast(FP16))

                        h_sb = wk.tile([128, FEAT], F32R, name="h_sb",
                                       tag="h")
                        nc.scalar.activation(h_sb[:], X_sb[:], AF.Copy,
                                             scale=rin2[:])
                        tpb = psA.tile([128, 1024], F32R, name="tpb", tag="big")
                        for ft in range(NF):
                            nc.tensor.transpose(
                                tpb[:, 128 * ft : 128 * (ft + 1)],
                                h_sb[:, 128 * ft : 128 * (ft + 1)],
                                ident_r,
                            )
                        hT = wk.tile([128, FEAT], F32R, name="hT", tag="hT")
                        nc.vector.tensor_copy(hT[:, 0:384], tpb[:, 0:384])
                        nc.scalar.copy(hT[:, 384:768], tpb[:, 384:768])
                        g_ps = psA.tile([128, 1024], F32, name="g_ps",
                                        tag="big")
                        for ft in range(NF):
                            MM(
                                g_ps[:, 0:512],
                                hT[:, 128 * ft : 128 * (ft + 1)],
                                wm_sb[:, FEAT * ft : FEAT * ft + 512],
                                start=(ft == 0),
                                stop=(ft == NF - 1),
                            )
                            MM(
                                g_ps[:, 512:768],
                                hT[:, 128 * ft : 128 * (ft + 1)],
                                wm_sb[:, FEAT * ft + 512 : FEAT * (ft + 1)],
                                start=(ft == 0),
                                stop=(ft == NF - 1),
                            )
                        P1 = wk.tile([128, FEAT], F32, name="P1", tag="P1")
                        nc.vector.tensor_copy(P1[:, 0:384], g_ps[:, 0:384])
                        nc.scalar.copy(P1[:, 384:768], g_ps[:, 384:768])
                        if not last:
                            p2_ps = psB.tile([128, 512], F32, name="p2_ps",
                                             tag="small")
                            for ft in range(NF):
                                MM(
                                    p2_ps[:, 0:384],
                                    hT[:, 128 * ft : 128 * (ft + 1)],
                                    wmqkv_sb[:, 384 * ft : 384 * (ft + 1)],
                                    start=(ft == 0),
                                    stop=(ft == NF - 1),
                                )
                            P2s = wk.tile([128, 384], F32, name="P2s",
                                          tag="P2s")
                            nc.scalar.copy(P2s[:], p2_ps[:, 0:384])

                        # ---- post-AG: m-major fp8 attention (no P^T
                        # transposes: scores computed as S^T chunks, exp
                        # writes P^T to SBUF directly; Z via pt_j^T @ ones) --
                        ago = ag_out.rearrange("(j r) c -> r j c", r=128)
                        ktf = wk.tile([128, SEQ], FP8, name="ktf", tag="ktf")
                        vf = wk.tile([128, SEQ], FP8, name="vf", tag="vf")
                        ktf_r = ktf.rearrange("r (j m) -> r j m", j=N_CORES)
                        nc.sync.dma_start(ktf_r[:, 0:4, :],
                                          ago[:, 0:4, 0:128].bitcast(FP8))
                        nc.sync.dma_start(ktf_r[:, 4:8, :],
                                          ago[:, 4:8, 0:128].bitcast(FP8))
                        nc.sync.dma_start(
                            vf.rearrange("r (j d) -> r j d", j=N_CORES),
                            ago[:, :, 128:256].bitcast(FP8),
                        )
                        vf16 = wk.tile([128, SEQ], FP16, name="vf16",
                                       tag="vf16")
                        nc.vector.tensor_copy(vf16[:, 0:512], vf[:, 0:512])
                        nc.gpsimd.tensor_copy(vf16[:, 512:1024],
                                              vf[:, 512:1024])
                        s_ps = psA.tile([128, 1024], F32, name="s_ps",
                                        tag="big")
                        for j in range(8):
                            MM(s_ps[:, 128 * j : 128 * (j + 1)],
                               ktf[:, 128 * j : 128 * (j + 1)], qt[:])
                        pt = wk.tile([128, SEQ], FP16, name="pt", tag="pt")
                        nc.scalar.activation(
                            pt[:, 0:512], s_ps[:, 0:512], AF.Exp,
                            scale=1.0 / (KSC * KSC),
                        )
                        nc.scalar.activation(
                            pt[:, 512:1024], s_ps[:, 512:1024], AF.Exp,
                            scale=1.0 / (KSC * KSC),
                        )
                        at_ps = psB.tile([128, 512], F32, name="at_ps",
                                         tag="small")
                        for j in range(8):
                            MM(
                                at_ps[:, 0:128],
                                vf16[:, 128 * j : 128 * (j + 1)],
                                pt[:, 128 * j : 128 * (j + 1)],
                                start=(j == 0),
                                stop=(j == 7),
                            )
                            MM(
                                tpk[:, 512:516].bitcast(F32),
                                pt[:, 128 * j : 128 * (j + 1)],
                                ones_f8[:],
                                start=(j == 0),
                                stop=(j == 7),
                            )
                        recip = wk.tile([128, 1], F32, name="recip", tag="sc4")
                        nc.vector.reciprocal(recip[:],
                                             tpk[:, 512:516].bitcast(F32))
                        at_sb = wk.tile([128, 128], FP16, name="at_sb",
                                        tag="at_sb")
                        nc.vector.tensor_copy(at_sb[:], at_ps[:, 0:128])

                    # ============ shared X / qkv_raw / rin2 update ============
                    # q2 MM first so its sem lands earliest (DVE unparks the
                    # most-recently-ready wait: X halves then win over qkv).
                    if not last:
                        q2_ps = psB.tile([128, 512], F32, name="q2_ps",
                                         tag="small")
                        MM(q2_ps[:, 0:384], at_sb[:], wowmq_sb[:])
                    x_ps = psA.tile([128, 1024], F32, name="x_ps", tag="big")
                    MM(x_ps[:, 0:512], at_sb[:], wowm_sb[:, 0:512])
                    MM(x_ps[:, 512:768], at_sb[:], wowm_sb[:, 512:768])
                    if not last:
                        qkv_raw = wk.tile([128, 384], F32, name="qkv_raw",
                                          tag="qraw")
                        nc.vector.scalar_tensor_tensor(
                            qkv_raw[:], q2_ps[:, 0:384], recip[:], P2s[:],
                            op0=MULT, op1=ADD,
                        )
                    X_sb = wk.tile([128, FEAT], F32, name="X_sb", tag="X")
                    nc.vector.scalar_tensor_tensor(
                        X_sb[:, 0:384], x_ps[:, 0:384], recip[:],
                        P1[:, 0:384], op0=MULT, op1=ADD,
                    )
                    nc.vector.scalar_tensor_tensor(
                        X_sb[:, 384:768], x_ps[:, 384:768], recip[:],
                        P1[:, 384:768], op0=MULT, op1=ADD,
                    )
                    sq2 = wk.tile([128, FEAT], F32, name="sq2", tag="sq")
                    ssa = wk.tile([128, 1], F32, name="ssa", tag="sc5")
                    ssb = wk.tile([128, 1], F32, name="ssb", tag="sc5b")
                    nc.scalar.activation(sq2[:, 0:384], X_sb[:, 0:384],
                                         AF.Square, accum_out=ssa[:])
                    nc.scalar.activation(sq2[:, 384:768], X_sb[:, 384:768],
                                         AF.Square, accum_out=ssb[:])
                    ss2 = wk.tile([128, 1], F32, name="ss2", tag="sc5c")
                    nc.vector.tensor_tensor(ss2[:], ssa[:], ssb[:], ADD)
                    nrm2 = wk.tile([128, 1], F32, name="nrm2", tag="sc6")
                    nc.scalar.activation(nrm2[:], ss2[:], AF.Sqrt)
                    nrm2c = wk.tile([128, 1], F32, name="nrm2c", tag="sc6b")
                    nc.vector.tensor_scalar_max(nrm2c[:], nrm2[:], 1e-12)
                    rin2 = wk.tile([128, 1], F32, name="rin2", tag="sc7")
                    nc.vector.reciprocal(rin2[:], nrm2c[:])
                    if not last:
                        kscale = wk.tile([128, 1], F32, name="kscale",
                                         tag="sc9")
                        nc.vector.tensor_scalar_mul(kscale[:], rin2[:], KSC)

                # ---- final h^T (fp16), AllGathered to all cores ----
                h_sb = wk.tile([128, FEAT], F32, name="h_sbf", tag="h")
                nc.scalar.activation(h_sb[:], X_sb[:], AF.Copy, scale=rin2[:])
                tpf = psA.tile([128, 1024], F32, name="tpf", tag="big")
                for ft in range(NF):
                    nc.tensor.transpose(
                        tpf[:, 128 * ft : 128 * (ft + 1)],
                        h_sb[:, 128 * ft : 128 * (ft + 1)],
                        ident_sb[:],
                    )
                hTf = wk.tile([128, FEAT], FP16, name="hTf", tag="hTf")
                nc.vector.tensor_copy(hTf[:, 0:384], tpf[:, 0:384])
                nc.scalar.copy(hTf[:, 384:768], tpf[:, 384:768])
                agh_in = dram.tile([FEAT, 128], FP16, bufs=1)
                nc.sync.dma_start(
                    agh_in.rearrange("(t p) m -> p t m", p=128),
                    hTf.rearrange("p (t m) -> p t m", t=NF),
                )
                agh_out = dram.tile(
                    [N_CORES * FEAT, 128], FP16, addr_space="Shared", bufs=1
                )
                nc.gpsimd.collective_compute(
                    "AllGather", mybir.AluOpType.bypass, replica_groups=rg,
                    ins=[agh_in.opt()], outs=[agh_out.opt()],
                )

            # ======================= out-projection ==========================
            with tc.tile_pool(name="oph", bufs=2) as op:
                htf_sb = op.tile([128, NF * SEQ], FP16, name="htf_sb", tag="htf",
                                 bufs=1)
                agh_r = agh_out.rearrange("(j t p) m -> p t j m", t=NF, p=128)
                for ft in range(NF):
                    nc.sync.dma_start(
                        htf_sb[:, SEQ * ft : SEQ * (ft + 1)].rearrange(
                            "p (j m) -> p j m", j=N_CORES
                        ),
                        agh_r[:, ft, :, :],
                    )

                wout_r = wout_h.rearrange("(t p) v -> p t v", p=128)
                for c in range(NVT):
                    woc = op.tile([128, NF * 128], FP16, name="woc", tag="woc",
                                  bufs=3)
                    nc.sync.dma_start(
                        woc.rearrange("p (t v) -> p t v", t=NF),
                        wout_r[:, :, 128 * c : 128 * (c + 1)],
                    )
                    L_ps = psA.tile([128, 1024], F32, name="L_ps", tag="big")
                    for ft in range(NF):
                        MM(
                            L_ps[:, 0:512],
                            woc[:, 128 * ft : 128 * (ft + 1)],
                            htf_sb[:, SEQ * ft : SEQ * ft + 512],
                            start=(ft == 0),
                            stop=(ft == NF - 1),
                        )
                        MM(
                            L_ps[:, 512:1024],
                            woc[:, 128 * ft : 128 * (ft + 1)],
                            htf_sb[:, SEQ * ft + 512 : SEQ * (ft + 1)],
                            start=(ft == 0),
                            stop=(ft == NF - 1),
                        )
                    l_sb = op.tile([128, SEQ], FP16, name="l_sb", tag="l_sb")
                    nc.scalar.copy(l_sb[:, 0:512], L_ps[:, 0:512])
                    nc.scalar.copy(l_sb[:, 512:1024], L_ps[:, 512:1024])

                    nc.vector.max(topv_all[:, RW * c : RW * c + 8], l_sb[:])
                    prev = l_sb
                    for r in range(1, rounds):
                        mrb = op.tile(
                            [128, SEQ], FP16, name="mrb", tag=f"mrb{r % 2}"
                        )
                        nc.vector.match_replace(
                            mrb[:],
                            topv_all[:, RW * c + 8 * (r - 1) : RW * c + 8 * r],
                            prev[:],
                            -60000.0,
                        )
                        nc.vector.max(
                            topv_all[:, RW * c + 8 * r : RW * c + 8 * (r + 1)],
                            mrb[:],
                        )
                        prev = mrb

                nc.sync.dma_start(
                    topv.rearrange("(c p) w -> p c w", p=128),
                    topv_all.rearrange("p (c w) -> p c w", c=NVT),
                )

    _split_excess_waits(nc)
    return nc


_CACHE = {}


def _get_program(nblocks, rounds):
    key = ("fast2", nblocks, rounds)
    if key not in _CACHE:
        _CACHE[key] = _build_fast2(nblocks, rounds)
    return _CACHE[key]


def kernel(x, pe, W_emb, b_emb, Wq, bq, Wk, bk, Wv, bv, Wo, bo, W1, b1, Wout,
           bout, k, _profile=False, _nblocks=NBLOCKS):
    x = np.asarray(x, dtype=np.float32).reshape(SEQ, VOCAB)
    pe = np.asarray(pe, dtype=np.float32)
    W_emb = np.asarray(W_emb, dtype=np.float32)
    Wq = np.asarray(Wq, dtype=np.float32)
    Wk = np.asarray(Wk, dtype=np.float32)
    Wv = np.asarray(Wv, dtype=np.float32)
    Wo = np.asarray(Wo, dtype=np.float32)
    W1 = np.asarray(W1, dtype=np.float32)
    Wout = np.asarray(Wout, dtype=np.float32)
    b_emb = np.asarray(b_emb, dtype=np.float32)
    bq = np.asarray(bq, dtype=np.float32)
    bk = np.asarray(bk, dtype=np.float32)
    bv = np.asarray(bv, dtype=np.float32)
    bo = np.asarray(bo, dtype=np.float32)
    b1 = np.asarray(b1, dtype=np.float32)
    bout = np.asarray(bout, dtype=np.float32)
    k = int(np.asarray(k))
    rounds = max(1, math.ceil(k / 8))
    assert rounds * 8 <= 24, f"k={k} too large for this kernel"
    assert not (np.any(bq) or np.any(bk) or np.any(bv) or np.any(bo)
                or np.any(b1) or np.any(bout)), "bias path not supported"

    nc = _get_program(_nblocks, rounds)

    # host-side shard prep
    VTOT = N_CORES * VP
    wemb_pad = np.zeros((VTOT, FEAT), dtype=np.float32)
    wemb_pad[:VOCAB, :] = W_emb
    wout_pad = np.zeros((FEAT, VTOT), dtype=np.float32)
    wout_pad[:, :VOCAB] = Wout
    wqkv = np.ascontiguousarray(np.concatenate([Wq, Wk, Wv], axis=1))
    ident = np.eye(128, dtype=np.float32)

    xT_pad = np.zeros((VTOT, SEQ), dtype=np.float32)
    xT_pad[:VOCAB, :] = x.T
    W1_64 = W1.astype(np.float64)
    Wm64 = W1_64 + W1_64 @ W1_64
    Wm = Wm64.astype(np.float32)
    Wmqkv64 = Wm64 @ wqkv.astype(np.float64)
    Wmqkv = Wmqkv64.astype(np.float32)
    Wo64 = Wo.astype(np.float64)
    WoWm = (Wo64 @ Wm64).astype(np.float32)
    WoWmqkv = (Wo64 @ Wmqkv64).astype(np.float32)
    ident_b = ident.astype(np.float16)

    in_maps = []
    for i in range(N_CORES):
        m = {
            "xT_h": np.ascontiguousarray(
                xT_pad[VP * i : VP * (i + 1), :]
            ).astype(np.float16),
            "wemb_h": np.ascontiguousarray(
                wemb_pad[VP * i : VP * (i + 1), :]
            ).astype(np.float16),
            "wqkv": wqkv,
            "wm": Wm,
            "wmqkv": Wmqkv,
            "wowm_h": WoWm.astype(np.float16),
            "wowmq_h": WoWmqkv.astype(np.float16),
            "wout_h": np.ascontiguousarray(
                wout_pad[:, VP * i : VP * (i + 1)]
            ).astype(np.float16),
            "pe_i": np.ascontiguousarray(
                np.concatenate(
                    [pe[64 * i : 64 * (i + 1), :],
                     pe[512 + 64 * i : 512 + 64 * (i + 1), :]], axis=0
                ) + b_emb
            ),
            "ident": ident,
            "ident_b": ident_b,
        }
        in_maps.append(m)

    res = None
    for attempt in range(3):
        try:
            res = run_bass_kernel_spmd(
                nc, in_maps, core_ids=list(range(N_CORES)), trace=_profile
            )
            break
        except Exception:
            # transient NRT/axon failures (e.g. NRT_EXEC_UNIT_UNRECOVERABLE)
            # have been observed; retry with the cached executable
            if attempt == 2:
                raise
            import time as _time
            _time.sleep(5)

    RW = 8 * rounds
    full = np.concatenate(
        [np.asarray(res.results[i]["topv"], dtype=np.float32).reshape(VP, RW)
         for i in range(N_CORES)], axis=0
    )
    vals = full[:VOCAB, :k]  # [VOCAB, k]
    out = np.ascontiguousarray(vals.T)[None, :, :]  # [1, k, VOCAB]

    if _profile:
        return out.astype(np.float32), res
    return out.astype(np.float32)
